# revision 31
# baseline (speedup 1.0000x reference)
"""Trainium2 Bass kernel for a dense transformer block.

Computes: ffwd(ln2(sa(ln1(x)) + x)) + x  (residual 2 connects to x)
with causal self-attention (6 heads, head_size 64), seq len 256, n_embed 384.

Sharding: data-parallel over batch (B=256) -> 32 items per NeuronCore,
weights replicated. All LN gains/biases and the softmax scale are folded
into the weight matrices on the host; matmul inputs are bf16 with fp32
PSUM accumulation; the LN/residual spine stays fp32.
"""

import sys
from contextlib import ExitStack

sys.path.insert(0, "/opt/trn_rl_repo")

import numpy as np
import ml_dtypes

import concourse.bass as bass
import concourse.tile as tile
from concourse import bacc, mybir
from concourse.bass_utils import run_bass_kernel_spmd

B, T, E, H, HS = 256, 256, 384, 6, 64
FF = 4 * E  # 1536
N_CORES = 8
IPC = B // N_CORES  # items per core
EPS = 1e-5

BF16 = mybir.dt.bfloat16
F32 = mybir.dt.float32
F8 = mybir.dt.float8e4
AF = mybir.ActivationFunctionType
OP = mybir.AluOpType
DR = mybir.MatmulPerfMode.DoubleRow
BF = ml_dtypes.bfloat16
F8NP = ml_dtypes.float8_e4m3

_CACHE = {}


def _setup_act_tables():
    """Force a single ACT table set (natural_log_exp_and_others) so walrus
    never thrashes between the exp and ln sets: we only use ln/exp/relu/
    copy/identity, which all live in that one set."""
    import os, json, tempfile

    if "BASS_ACT_ROOT_JSON_PATH" in os.environ:
        return
    from neuronxcc.driver.Job import Job
    from neuronxcc.driver.jobs.support.FindActInfo import findActInfoFile

    src = findActInfoFile(Job.getPackageDir(), "gen3")
    d = json.load(open(src))
    keep = [s for s in d["act_func_sets"] if s["name"] == "natural_log_exp_and_others"]
    assert keep, "natural_log_exp_and_others set not found"
    d["act_func_sets"] = keep
    dst_dir = tempfile.mkdtemp(prefix="act_custom_")
    srcdir = os.path.dirname(src)
    for key in d["pwp_file_keys"]:
        fn = keep[0][key]
        os.symlink(os.path.join(srcdir, fn), os.path.join(dst_dir, fn))
    dst = os.path.join(dst_dir, "act_info.json")
    with open(dst, "w") as f:
        json.dump(d, f)
    os.environ["BASS_ACT_ROOT_JSON_PATH"] = dst

    # Bacc's insert_act_table_loads must agree with walrus on set ids:
    # filter its table view to the same single set (id 0).
    import concourse.hw_specs as hw_specs
    import concourse.bacc as bacc_mod

    orig = hw_specs.get_activation_tables

    def filtered(arch):
        t = orig(arch)
        return {"natural_log_exp_and_others": t["natural_log_exp_and_others"]}

    hw_specs.get_activation_tables = filtered
    bacc_mod.get_activation_tables = filtered


def _build():
    _setup_act_tables()
    nc = bacc.Bacc("TRN2", target_bir_lowering=False, debug=False)

    x_d = nc.dram_tensor("x", [IPC, T, E], BF16, kind="ExternalInput").ap()
    # q/k/v/ffn1 weights: features 0-255 as fp8 DoubleRow planes, 256-383 bf16
    wq8_d = nc.dram_tensor("wq8", [128, 2, E], F8, kind="ExternalInput").ap()
    wk8_d = nc.dram_tensor("wk8", [128, 2, E], F8, kind="ExternalInput").ap()
    wv8_d = nc.dram_tensor("wv8", [128, 2, E], F8, kind="ExternalInput").ap()
    wqb_d = nc.dram_tensor("wqb", [128, E], BF16, kind="ExternalInput").ap()
    wkb_d = nc.dram_tensor("wkb", [128, E], BF16, kind="ExternalInput").ap()
    wvb_d = nc.dram_tensor("wvb", [128, E], BF16, kind="ExternalInput").ap()
    wo_d = nc.dram_tensor("wo", [E, E], BF16, kind="ExternalInput").ap()
    w1f8_d = nc.dram_tensor("w1f8", [128, 2, FF], F8, kind="ExternalInput").ap()
    w1b_d = nc.dram_tensor("w1b", [128, FF], BF16, kind="ExternalInput").ap()
    w2f8_d = nc.dram_tensor("w2f8", [6, 128, 2, E], F8, kind="ExternalInput").ap()
    # biases pre-laid-out on the host: [128, n] column tiles / [128, E] rows
    bq_d = nc.dram_tensor("bq", [128, 3], F32, kind="ExternalInput").ap()
    bk_d = nc.dram_tensor("bk", [128, 3], F32, kind="ExternalInput").ap()
    b1_d = nc.dram_tensor("b1", [128, 12], F32, kind="ExternalInput").ap()
    bo_d = nc.dram_tensor("bo", [128, E], F32, kind="ExternalInput").ap()
    b2_d = nc.dram_tensor("b2", [128, E], F32, kind="ExternalInput").ap()
    mask_d = nc.dram_tensor("maskt", [128, 128], BF16, kind="ExternalInput").ap()
    ones_d = nc.dram_tensor("ones64", [128, 64], BF16, kind="ExternalInput").ap()
    out_d = nc.dram_tensor("out", [IPC, T, E], F32, kind="ExternalOutput").ap()

    with tile.TileContext(nc) as tc, ExitStack() as ctx:
        singles = ctx.enter_context(tc.tile_pool(name="singles", bufs=1))
        p_x = ctx.enter_context(tc.tile_pool(name="p_x", bufs=16))
        p_h = ctx.enter_context(tc.tile_pool(name="p_h", bufs=8))
        p_T = ctx.enter_context(tc.tile_pool(name="p_T", bufs=2))
        p_h1T = ctx.enter_context(tc.tile_pool(name="p_h1T", bufs=4))
        p_relu = ctx.enter_context(tc.tile_pool(name="p_relu", bufs=2))
        p_xb = ctx.enter_context(tc.tile_pool(name="p_xb", bufs=8))
        p_rec = ctx.enter_context(tc.tile_pool(name="p_rec", bufs=4))
        p_qk = ctx.enter_context(tc.tile_pool(name="p_qk", bufs=3))
        p_e = ctx.enter_context(tc.tile_pool(name="p_e", bufs=2))
        p_sm = ctx.enter_context(tc.tile_pool(name="p_sm", bufs=16))
        p_y = ctx.enter_context(tc.tile_pool(name="p_y", bufs=6))

        pp_big = ctx.enter_context(tc.tile_pool(name="pp_big", bufs=2, space="PSUM"))
        pp_v = ctx.enter_context(tc.tile_pool(name="pp_v", bufs=2, space="PSUM"))
        pp_s = ctx.enter_context(tc.tile_pool(name="pp_s", bufs=2, space="PSUM"))
        pp_do = ctx.enter_context(tc.tile_pool(name="pp_do", bufs=2, space="PSUM"))

        # ---- constants / weights (emission deferred via load_weights so the
        # first pair's x DMAs + LN1 go out ahead of the bulk weight traffic) ----
        w8_sb = {}
        wb_sb = {}
        wo_sb = []
        w2_sb = []

        def load_weights():
            # qkv first (stage_a2(0) needs them soonest), then wo/w1/w2
            for nm, src8, srcb in (
                ("q", wq8_d, wqb_d), ("k", wk8_d, wkb_d), ("v", wv8_d, wvb_d)
            ):
                t8 = singles.tile([128, 2, E], F8, tag=f"w8_{nm}")
                nc.scalar.dma_start(t8[:], src8[:])
                w8_sb[nm] = t8
                tb = singles.tile([128, E], BF16, tag=f"wb_{nm}")
                nc.scalar.dma_start(tb[:], srcb[:])
                wb_sb[nm] = tb
            for kt in range(3):
                t = singles.tile([128, E], BF16, tag=f"wo_{kt}")
                nc.scalar.dma_start(t[:], wo_d[kt * 128 : (kt + 1) * 128, :])
                wo_sb.append(t)
            t8 = singles.tile([128, 2, FF], F8, tag="w8_1")
            nc.scalar.dma_start(t8[:], w1f8_d[:])
            w8_sb["1"] = t8
            tb = singles.tile([128, FF], BF16, tag="wb_1")
            nc.scalar.dma_start(tb[:], w1b_d[:])
            wb_sb["1"] = tb
            for kt in range(6):
                t = singles.tile([128, 2, E], F8, tag=f"w2_{kt}")
                nc.scalar.dma_start(t[:], w2f8_d[kt])
                w2_sb.append(t)

        bq_sb = singles.tile([128, 3], F32, tag="bq")
        bk_sb = singles.tile([128, 3], F32, tag="bk")
        b1_sb = singles.tile([128, 12], F32, tag="b1")
        bo_bc = singles.tile([128, E], F32, tag="bo_bc")
        b2_bc = singles.tile([128, E], F32, tag="b2_bc")
        maskt = singles.tile([128, 128], BF16, tag="maskt")
        ones64 = singles.tile([128, 64], BF16, tag="ones64")

        def load_biases():
            for t, srcd in ((bq_sb, bq_d), (bk_sb, bk_d), (b1_sb, b1_d),
                            (bo_bc, bo_d), (b2_bc, b2_d), (maskt, mask_d),
                            (ones64, ones_d)):
                nc.scalar.dma_start(t[:], srcd[:])

        eps_t = singles.tile([128, 1], F32, tag="eps")
        nc.vector.memset(eps_t[:], EPS)

        def layernorm_quad(pairs):
            """For each (src, dst) in pairs (up to 4): dst (bf16) =
            (src - mean) * rsqrt(var + eps) row-wise over 384.
            The rsqrt is exp(-0.5*ln(var+eps)) batched over all tiles so the
            per-op ACT overhead is paid once, and only the exp/ln table set
            is ever touched."""
            n = len(pairs)
            mv_all = p_sm.tile([128, n, 2], F32, tag="bnmv")
            for j, (src, _) in enumerate(pairs):
                st = p_sm.tile([128, 6], F32, tag="bnst")
                nc.vector.bn_stats(st[:], src[:])
                nc.vector.bn_aggr(mv_all[:, j, :], st[:])
            lnv = p_sm.tile([128, n], F32, tag="lnv")
            nc.scalar.activation(lnv[:], mv_all[:, :, 1], AF.Ln, bias=eps_t[:])
            rstd = p_sm.tile([128, n], F32, tag="rstd")
            nc.scalar.activation(rstd[:], lnv[:], AF.Exp, scale=-0.5)
            for j, (src, dst) in enumerate(pairs):
                nc.vector.tensor_scalar(
                    dst[:], src[:], mv_all[:, j, 0:1], rstd[:, j : j + 1],
                    op0=OP.subtract, op1=OP.mult,
                )

        # ---- main loop: 16 pairs of batch items, software-pipelined ----
        # Stage A (load, LN1, transpose, QKV) of pair pp+1 is emitted before
        # stage B (attention, sa, LN2, FFN, out) of pair pp so the PE always
        # has independent matmul work during B's LN2 serial chain (keeps the
        # HAM clock gate warm).

        def stage_a1(pp):
            x_sb = [[None, None], [None, None]]
            h1_sb = [[None, None], [None, None]]
            ln_pairs = []
            for it in range(2):
                i = 2 * pp + it
                for tt in range(2):
                    xt = p_x.tile([128, E], BF16, tag="x")
                    nc.sync.dma_start(
                        xt[:], x_d[i, tt * 128 : (tt + 1) * 128, :]
                    )
                    x_sb[it][tt] = xt
                    h1 = p_h.tile([128, E], BF16, tag="h1")
                    ln_pairs.append((xt, h1))
                    h1_sb[it][tt] = h1
            layernorm_quad(ln_pairs)

            # transpose h1 -> h1T [E, 2*T] (feature-major) via DMA xbar
            h1T = p_h1T.tile([128, 3, 512], BF16, tag="h1T")
            for it in range(2):
                for tt in range(2):
                    c0 = it * 256 + tt * 128
                    nc.sync.dma_start_transpose(
                        out=h1T[:, :, c0 : c0 + 128], in_=h1_sb[it][tt][:]
                    )
            # fp8 copy of feature planes 0,1 for the DoubleRow matmuls
            h1T8 = p_h1T.tile([128, 2, 512], F8, tag="h1T8")
            nc.vector.tensor_copy(h1T8[:], h1T[:, 0:2, :])
            return dict(x_sb=x_sb, h1T=h1T, h1T8=h1T8)

        def stage_a2(pp, st):
            h1T, h1T8 = st["h1T"], st["h1T8"]
            # qT, kT projections (feature-major): qT[f, t] over both items.
            # Weights are prescaled x64 on the host (fp8 normal range); the
            # eviction's free activation scale undoes it exactly.
            qT = p_qk.tile([128, 3, 512], BF16, tag="qT")
            kT = p_qk.tile([128, 3, 512], BF16, tag="kT")
            for dst, w8, wb, b_sb in (
                (qT, w8_sb["q"], wb_sb["q"], bq_sb),
                (kT, w8_sb["k"], wb_sb["k"], bk_sb),
            ):
                for ft in range(3):
                    ps = pp_big.tile([128, 512], F32, tag="ps_big")
                    nc.tensor.matmul(
                        ps[:], wb[:, ft * 128 : (ft + 1) * 128], h1T[:, 2, :],
                        start=True, stop=False,
                    )
                    nc.tensor.matmul(
                        ps[:], w8[:, :, ft * 128 : (ft + 1) * 128], h1T8[:],
                        start=False, stop=True, perf_mode=DR,
                    )
                    nc.scalar.activation(
                        dst[:, ft, :], ps[:], AF.Identity,
                        bias=b_sb[:, ft : ft + 1], scale=1.0 / 64,
                    )

            # v (token-major): v[t, f] per item
            v_sb = [None, None]
            for it in range(2):
                vt = p_h.tile([128, 2, E], BF16, tag="v")
                for tt in range(2):
                    c0 = it * 256 + tt * 128
                    ps = pp_v.tile([128, E], F32, tag="ps_v")
                    nc.tensor.matmul(
                        ps[:], h1T[:, 2, c0 : c0 + 128], wb_sb["v"][:],
                        start=True, stop=False,
                    )
                    nc.tensor.matmul(
                        ps[:], h1T8[:, :, c0 : c0 + 128], w8_sb["v"][:],
                        start=False, stop=True, perf_mode=DR,
                    )
                    nc.scalar.activation(vt[:, tt, :], ps[:], AF.Identity, scale=1.0 / 64)
                v_sb[it] = vt
            st["qT"], st["kT"], st["v_sb"] = qT, kT, v_sb

        def stage_b1(pp, st):
            qT, kT, v_sb = st["qT"], st["kT"], st["v_sb"]
            # residual bases with biases folded in (on the idle GPSIMD):
            # xbo = x + bo' (feeds y), xb2 = x + b2 (feeds final out)
            xbo = [[None, None], [None, None]]
            xb2 = [[None, None], [None, None]]
            for it in range(2):
                for tt in range(2):
                    a = p_xb.tile([128, E], F32, tag="xbo")
                    nc.gpsimd.tensor_add(a[:], st["x_sb"][it][tt][:], bo_bc[:])
                    xbo[it][tt] = a
                    b = p_xb.tile([128, E], F32, tag="xb2")
                    nc.gpsimd.tensor_add(b[:], st["x_sb"][it][tt][:], b2_bc[:])
                    xb2[it][tt] = b
            st["xbo"], st["xb2"] = xbo, xb2
            # attention, transposed layout: scoresT[k, q] per (item, head)
            attnT = p_T.tile([128, 3, 512], BF16, tag="attnT")
            for it in range(2):
                c0 = it * 256
                expA = p_e.tile([128, 6, 384], BF16, tag="expA")
                for h in range(H):
                    p3 = h // 2
                    r0 = 64 * (h % 2)
                    qh = qT[r0 : r0 + 64, p3, c0 : c0 + 256]
                    ps_s = pp_s.tile([128, 384], F32, tag="ps_s")
                    # k-tile 0: all q columns
                    nc.tensor.matmul(
                        ps_s[:, 0:256],
                        kT[r0 : r0 + 64, p3, c0 : c0 + 128],
                        qh,
                        start=True,
                        stop=True,
                        skip_group_check=True,
                    )
                    # k-tile 1: only q columns 128:256 (rest fully masked)
                    nc.tensor.matmul(
                        ps_s[:, 256:384],
                        kT[r0 : r0 + 64, p3, c0 + 128 : c0 + 256],
                        qT[r0 : r0 + 64, p3, c0 + 128 : c0 + 256],
                        start=True,
                        stop=True,
                        skip_group_check=True,
                    )
                    # one fused exp over both score blocks
                    nc.scalar.activation(expA[:, h, :], ps_s[:], AF.Exp)
                    if h % 2 == 1:
                        # causal masks for this head pair: diag block of
                        # k-tile 0 (q 0:128) and the k-tile 1 block
                        # (on GpSimd: frees DVE for the LN2/eviction chains)
                        hp = h // 2
                        nc.vector.tensor_tensor(
                            expA[:, 2 * hp : 2 * hp + 2, 0:128],
                            expA[:, 2 * hp : 2 * hp + 2, 0:128],
                            maskt[:, None, :].to_broadcast((128, 2, 128)),
                            op=OP.mult,
                        )
                        nc.vector.tensor_tensor(
                            expA[:, 2 * hp : 2 * hp + 2, 256:384],
                            expA[:, 2 * hp : 2 * hp + 2, 256:384],
                            maskt[:, None, :].to_broadcast((128, 2, 128)),
                            op=OP.mult,
                        )

                # head pairs (2h, 2h+1) share a partition tile: pack their
                # denominator and output matmuls into one [128, 256] psum via
                # col tile_position, halving the DVE normalize work.
                for hp in range(3):
                    ps_d = pp_do.tile([128, 256], F32, tag="ps_do")
                    ps_o = pp_do.tile([128, 256], F32, tag="ps_do")
                    for sub in range(2):
                        h = 2 * hp + sub
                        r0 = 64 * sub
                        tp = (0, r0)
                        e0 = expA[:, h, 0:256]
                        e1 = expA[:, h, 256:384]
                        nc.tensor.matmul(
                            ps_d[r0 : r0 + 64, :], ones64[:], e0,
                            start=True, stop=False,
                            skip_group_check=True, tile_position=tp,
                        )
                        nc.tensor.matmul(
                            ps_d[r0 : r0 + 64, 128:256], ones64[:], e1,
                            start=False, stop=True,
                            skip_group_check=True, tile_position=tp,
                        )
                        nc.tensor.matmul(
                            ps_o[r0 : r0 + 64, :],
                            v_sb[it][:, 0, 64 * h : 64 * h + 64], e0,
                            start=True, stop=False,
                            skip_group_check=True, tile_position=tp,
                        )
                        nc.tensor.matmul(
                            ps_o[r0 : r0 + 64, 128:256],
                            v_sb[it][:, 1, 64 * h : 64 * h + 64], e1,
                            start=False, stop=True,
                            skip_group_check=True, tile_position=tp,
                        )
                    rec = p_rec.tile([128, 256], F32, tag="rec")
                    nc.vector.reciprocal_approx_fast(rec[:], ps_d[:])
                    nc.vector.tensor_tensor(
                        attnT[:, hp, c0 : c0 + 256], ps_o[:], rec[:], op=OP.mult,
                    )

            st["attnT"] = attnT

        def warm(n=1):
            for _ in range(n):
                pw = pp_v.tile([64, 384], F32, tag="ps_v")
                nc.tensor.matmul(
                    pw[:], ones64[:], wo_sb[0][:], start=True, stop=True,
                    skip_group_check=True,
                )

        def stage_b2a(pp, st):
            attnT = st["attnT"]
            # sa (token-major) + residual + bo, then LN2
            h2_sb = [[None, None], [None, None]]
            ln2_pairs = []
            for it in range(2):
                for tt in range(2):
                    ps = pp_v.tile([128, E], F32, tag="ps_v")
                    for kt in range(3):
                        nc.tensor.matmul(
                            ps[:],
                            attnT[:, kt, it * 256 + tt * 128 : it * 256 + tt * 128 + 128],
                            wo_sb[kt][:],
                            start=(kt == 0),
                            stop=(kt == 2),
                        )
                    y = p_y.tile([128, E], F32, tag="y")
                    nc.vector.tensor_add(y[:], ps[:], st["xbo"][it][tt][:])
                    h2 = p_h.tile([128, E], BF16, tag="h2")
                    ln2_pairs.append((y, h2))
                    h2_sb[it][tt] = h2

            layernorm_quad(ln2_pairs)
            warm(3)

            # transpose h2 -> h2T via DMA xbar (PE transpose-mode does not
            # count as HAM activity and was re-throttling the PE clock)
            h2T = p_T.tile([128, 3, 512], BF16, tag="h2T")
            h2T8 = p_T.tile([128, 2, 512], F8, tag="h2T8")
            for it in range(2):
                for tt in range(2):
                    c0 = it * 256 + tt * 128
                    nc.sync.dma_start_transpose(
                        out=h2T[:, :, c0 : c0 + 128], in_=h2_sb[it][tt][:]
                    )
            nc.vector.tensor_copy(h2T8[:], h2T[:, 0:2, :])
            st["h2T"], st["h2T8"] = h2T, h2T8

        def stage_b2b(pp, st):
            h2T, h2T8 = st["h2T"], st["h2T8"]
            # FFN: reluT[c, t] = relu(W1'^T @ h2T + b1'), feature-major.
            # fp8 DoubleRow over features 0-255 + bf16 tail; w1/w2 quantized
            # at natural scale so both eviction engines need no rescale.
            relu_sb = p_relu.tile([128, 6, 2, 512], F8, tag="relu")
            for mt in range(12):
                ps = pp_big.tile([128, 512], F32, tag="ps_big")
                for it in range(2):
                    c0 = it * 256
                    nc.tensor.matmul(
                        ps[:, c0 : c0 + 256],
                        wb_sb["1"][:, mt * 128 : (mt + 1) * 128],
                        h2T[:, 2, c0 : c0 + 256],
                        start=True, stop=False,
                        skip_group_check=True,
                    )
                    nc.tensor.matmul(
                        ps[:, c0 : c0 + 256],
                        w8_sb["1"][:, :, mt * 128 : (mt + 1) * 128],
                        h2T8[:, :, c0 : c0 + 256],
                        start=False, stop=True, perf_mode=DR,
                        skip_group_check=True,
                    )
                dst = relu_sb[:, mt // 2, mt % 2, :]
                if mt % 2 == 0:
                    nc.scalar.activation(
                        dst, ps[:], AF.Relu, bias=b1_sb[:, mt : mt + 1]
                    )
                else:
                    # split evictions across ScalarE/DVE: the serial relu
                    # chain paces FFN2 otherwise
                    nc.vector.tensor_scalar(
                        dst, ps[:], b1_sb[:, mt : mt + 1], 0.0,
                        op0=OP.add, op1=OP.max,
                    )

            # ff (token-major) + residual to x + b2, DMA out
            for it in range(2):
                i = 2 * pp + it
                for tt in range(2):
                    c0 = it * 256 + tt * 128
                    ps = pp_v.tile([128, E], F32, tag="ps_v")
                    for kt in range(6):
                        nc.tensor.matmul(
                            ps[:],
                            relu_sb[:, kt, :, c0 : c0 + 128],
                            w2_sb[kt][:],
                            start=(kt == 0),
                            stop=(kt == 5),
                            perf_mode=DR,
                        )
                    ot = p_y.tile([128, E], F32, tag="ot")
                    nc.vector.tensor_add(ot[:], ps[:], st["xb2"][it][tt][:])
                    nc.sync.dma_start(out_d[i, tt * 128 : (tt + 1) * 128, :], ot[:])

        n_pairs = IPC // 2
        sts = {}
        sts[0] = stage_a1(0)
        load_weights()
        load_biases()
        for k in range(1, min(3, n_pairs)):
            sts[k] = stage_a1(k)
        for k in range(min(2, n_pairs)):
            stage_a2(k, sts[k])
        stage_b1(0, sts[0])
        # b1(pp+1) emitted between b2a(pp) and b2b(pp): engines are strict
        # FIFOs, so attention(pp+1)'s matmuls must sit *between* outproj(pp)
        # and FFN1(pp) in the PE stream to fill the h2T-transpose wait
        for pp in range(n_pairs):
            if pp + 3 < n_pairs:
                sts[pp + 3] = stage_a1(pp + 3)
            if pp + 2 < n_pairs:
                stage_a2(pp + 2, sts[pp + 2])
            stage_b2a(pp, sts[pp])
            if pp + 1 < n_pairs:
                stage_b1(pp + 1, sts[pp + 1])
            stage_b2b(pp, sts.pop(pp))

    nc.compile()
    return nc


def _prep_inputs(inputs):
    f = lambda v: np.asarray(v, dtype=np.float32)
    x = f(inputs["x"])
    Wq, Wk, Wv, Wo = f(inputs["Wq"]), f(inputs["Wk"]), f(inputs["Wv"]), f(inputs["Wo"])
    bo = f(inputs["bo"])
    W1, b1, W2, b2 = f(inputs["W1"]), f(inputs["b1"]), f(inputs["W2"]), f(inputs["b2"])
    g1, be1 = f(inputs["g1"]), f(inputs["be1"])
    g2, be2 = f(inputs["g2"]), f(inputs["be2"])

    scale = HS ** -0.5
    wq = (g1[:, None] * Wq) * scale
    bq = ((be1 @ Wq) * scale).astype(np.float32)
    wk = g1[:, None] * Wk
    bk = (be1 @ Wk).astype(np.float32)
    wv = g1[:, None] * Wv
    bv = be1 @ Wv
    # bv folded through Wo (softmax rows sum to 1): bo' = bv @ Wo + bo
    bo_p = (bv @ Wo + bo).astype(np.float32)
    w1 = g2[:, None] * W1
    b1_p = (be2 @ W1 + b1).astype(np.float32)

    def q8(a):
        return np.clip(a, -240.0, 240.0).astype(F8NP)

    def pack2(a):  # [256, M] -> [128, 2, M] DoubleRow planes
        return np.ascontiguousarray(a.reshape(2, 128, -1).transpose(1, 0, 2))

    col128 = lambda v: np.ascontiguousarray(v.reshape(-1, 128).T.astype(np.float32))
    row128 = lambda v: np.ascontiguousarray(
        np.broadcast_to(v.astype(np.float32), (128, v.shape[0]))
    )
    common = {
        # q/k/v prescaled x64 (fp8 normal range); undone by eviction scale
        "wq8": q8(pack2(wq[:256] * 64)), "wqb": (wq[256:] * 64).astype(BF),
        "wk8": q8(pack2(wk[:256] * 64)), "wkb": (wk[256:] * 64).astype(BF),
        "wv8": q8(pack2(wv[:256] * 64)), "wvb": (wv[256:] * 64).astype(BF),
        "wo": Wo.astype(BF),
        "w1f8": q8(pack2(w1[:256])), "w1b": w1[256:].astype(BF),
        "w2f8": q8(
            np.ascontiguousarray(W2.reshape(6, 2, 128, E).transpose(0, 2, 1, 3))
        ),
        "bq": col128(bq), "bk": col128(bk), "b1": col128(b1_p),
        "bo": row128(bo_p), "b2": row128(b2.astype(np.float32)),
        "maskt": np.triu(np.ones((128, 128), np.float32)).astype(BF),
        "ones64": np.ones((128, 64), np.float32).astype(BF),
    }
    in_maps = []
    for c in range(N_CORES):
        m = dict(common)
        m["x"] = np.ascontiguousarray(x[c * IPC : (c + 1) * IPC]).astype(BF)
        in_maps.append(m)
    return in_maps


def kernel(**inputs):
    if "nc" not in _CACHE:
        _CACHE["nc"] = _build()
    nc = _CACHE["nc"]
    in_maps = _prep_inputs(inputs)
    res = run_bass_kernel_spmd(nc, in_maps, core_ids=list(range(N_CORES)))
    _CACHE["last_result"] = res
    return np.concatenate([r["out"] for r in res.results], axis=0)



# revision 32
# speedup vs baseline: 1.1587x; 1.1587x over previous
"""Trainium2 Bass kernel for a dense transformer block.

Computes: ffwd(ln2(sa(ln1(x)) + x)) + x  (residual 2 connects to x)
with causal self-attention (6 heads, head_size 64), seq len 256, n_embed 384.

Sharding: data-parallel over batch (B=256) -> 32 items per NeuronCore,
weights replicated. All LN gains/biases and the softmax scale are folded
into the weight matrices on the host; matmul inputs are bf16 with fp32
PSUM accumulation; the LN/residual spine stays fp32.
"""

import sys
from contextlib import ExitStack

sys.path.insert(0, "/opt/trn_rl_repo")

import numpy as np
import ml_dtypes

import concourse.bass as bass
import concourse.tile as tile
from concourse import bacc, mybir
from concourse.bass_utils import run_bass_kernel_spmd

B, T, E, H, HS = 256, 256, 384, 6, 64
FF = 4 * E  # 1536
N_CORES = 8
IPC = B // N_CORES  # items per core
EPS = 1e-5

BF16 = mybir.dt.bfloat16
F32 = mybir.dt.float32
F8 = mybir.dt.float8e4
AF = mybir.ActivationFunctionType
OP = mybir.AluOpType
DR = mybir.MatmulPerfMode.DoubleRow
BF = ml_dtypes.bfloat16
F8NP = ml_dtypes.float8_e4m3

_CACHE = {}


def _setup_act_tables():
    """Force a single ACT table set (natural_log_exp_and_others) so walrus
    never thrashes between the exp and ln sets: we only use ln/exp/relu/
    copy/identity, which all live in that one set."""
    import os, json, tempfile

    if "BASS_ACT_ROOT_JSON_PATH" in os.environ:
        return
    from neuronxcc.driver.Job import Job
    from neuronxcc.driver.jobs.support.FindActInfo import findActInfoFile

    src = findActInfoFile(Job.getPackageDir(), "gen3")
    d = json.load(open(src))
    keep = [s for s in d["act_func_sets"] if s["name"] == "natural_log_exp_and_others"]
    assert keep, "natural_log_exp_and_others set not found"
    d["act_func_sets"] = keep
    dst_dir = tempfile.mkdtemp(prefix="act_custom_")
    srcdir = os.path.dirname(src)
    for key in d["pwp_file_keys"]:
        fn = keep[0][key]
        os.symlink(os.path.join(srcdir, fn), os.path.join(dst_dir, fn))
    dst = os.path.join(dst_dir, "act_info.json")
    with open(dst, "w") as f:
        json.dump(d, f)
    os.environ["BASS_ACT_ROOT_JSON_PATH"] = dst

    # Bacc's insert_act_table_loads must agree with walrus on set ids:
    # filter its table view to the same single set (id 0).
    import concourse.hw_specs as hw_specs
    import concourse.bacc as bacc_mod

    orig = hw_specs.get_activation_tables

    def filtered(arch):
        t = orig(arch)
        return {"natural_log_exp_and_others": t["natural_log_exp_and_others"]}

    hw_specs.get_activation_tables = filtered
    bacc_mod.get_activation_tables = filtered


def _build():
    _setup_act_tables()
    nc = bacc.Bacc("TRN2", target_bir_lowering=False, debug=False)

    x_d = nc.dram_tensor("x", [IPC, T, E], BF16, kind="ExternalInput").ap()
    # q/k/v/ffn1 weights: features 0-255 as fp8 DoubleRow planes, 256-383 bf16
    wq8_d = nc.dram_tensor("wq8", [128, 2, E], F8, kind="ExternalInput").ap()
    wk8_d = nc.dram_tensor("wk8", [128, 2, E], F8, kind="ExternalInput").ap()
    wv8_d = nc.dram_tensor("wv8", [128, 2, E], F8, kind="ExternalInput").ap()
    wqb_d = nc.dram_tensor("wqb", [128, E], BF16, kind="ExternalInput").ap()
    wkb_d = nc.dram_tensor("wkb", [128, E], BF16, kind="ExternalInput").ap()
    wvb_d = nc.dram_tensor("wvb", [128, E], BF16, kind="ExternalInput").ap()
    wo_d = nc.dram_tensor("wo", [E, E], BF16, kind="ExternalInput").ap()
    w1f8_d = nc.dram_tensor("w1f8", [128, 2, FF], F8, kind="ExternalInput").ap()
    w1b_d = nc.dram_tensor("w1b", [128, FF], BF16, kind="ExternalInput").ap()
    w2f8_d = nc.dram_tensor("w2f8", [6, 128, 2, E], F8, kind="ExternalInput").ap()
    # biases pre-laid-out on the host: [128, n] column tiles / [128, E] rows
    bq_d = nc.dram_tensor("bq", [128, 3], F32, kind="ExternalInput").ap()
    bk_d = nc.dram_tensor("bk", [128, 3], F32, kind="ExternalInput").ap()
    b1_d = nc.dram_tensor("b1", [128, 12], F32, kind="ExternalInput").ap()
    bo_d = nc.dram_tensor("bo", [128, E], F32, kind="ExternalInput").ap()
    b2_d = nc.dram_tensor("b2", [128, E], F32, kind="ExternalInput").ap()
    mask_d = nc.dram_tensor("maskt", [128, 128], BF16, kind="ExternalInput").ap()
    ones_d = nc.dram_tensor("ones64", [128, 64], BF16, kind="ExternalInput").ap()
    out_d = nc.dram_tensor("out", [IPC, T, E], BF16, kind="ExternalOutput").ap()

    with tile.TileContext(nc) as tc, ExitStack() as ctx:
        singles = ctx.enter_context(tc.tile_pool(name="singles", bufs=1))
        p_x = ctx.enter_context(tc.tile_pool(name="p_x", bufs=16))
        p_h = ctx.enter_context(tc.tile_pool(name="p_h", bufs=8))
        p_T = ctx.enter_context(tc.tile_pool(name="p_T", bufs=2))
        p_h1T = ctx.enter_context(tc.tile_pool(name="p_h1T", bufs=4))
        p_relu = ctx.enter_context(tc.tile_pool(name="p_relu", bufs=2))
        p_xb = ctx.enter_context(tc.tile_pool(name="p_xb", bufs=8))
        p_rec = ctx.enter_context(tc.tile_pool(name="p_rec", bufs=4))
        p_qk = ctx.enter_context(tc.tile_pool(name="p_qk", bufs=3))
        p_e = ctx.enter_context(tc.tile_pool(name="p_e", bufs=2))
        p_sm = ctx.enter_context(tc.tile_pool(name="p_sm", bufs=16))
        p_y = ctx.enter_context(tc.tile_pool(name="p_y", bufs=6))

        pp_big = ctx.enter_context(tc.tile_pool(name="pp_big", bufs=2, space="PSUM"))
        pp_v = ctx.enter_context(tc.tile_pool(name="pp_v", bufs=2, space="PSUM"))
        pp_s = ctx.enter_context(tc.tile_pool(name="pp_s", bufs=2, space="PSUM"))
        pp_do = ctx.enter_context(tc.tile_pool(name="pp_do", bufs=2, space="PSUM"))

        # ---- constants / weights (emission deferred via load_weights so the
        # first pair's x DMAs + LN1 go out ahead of the bulk weight traffic) ----
        w8_sb = {}
        wb_sb = {}
        wo_sb = []
        w2_sb = []

        def load_weights():
            # qkv first (stage_a2(0) needs them soonest), then wo/w1/w2
            for nm, src8, srcb in (
                ("q", wq8_d, wqb_d), ("k", wk8_d, wkb_d), ("v", wv8_d, wvb_d)
            ):
                t8 = singles.tile([128, 2, E], F8, tag=f"w8_{nm}")
                nc.scalar.dma_start(t8[:], src8[:])
                w8_sb[nm] = t8
                tb = singles.tile([128, E], BF16, tag=f"wb_{nm}")
                nc.scalar.dma_start(tb[:], srcb[:])
                wb_sb[nm] = tb
            for kt in range(3):
                t = singles.tile([128, E], BF16, tag=f"wo_{kt}")
                nc.scalar.dma_start(t[:], wo_d[kt * 128 : (kt + 1) * 128, :])
                wo_sb.append(t)
            t8 = singles.tile([128, 2, FF], F8, tag="w8_1")
            nc.scalar.dma_start(t8[:], w1f8_d[:])
            w8_sb["1"] = t8
            tb = singles.tile([128, FF], BF16, tag="wb_1")
            nc.scalar.dma_start(tb[:], w1b_d[:])
            wb_sb["1"] = tb
            for kt in range(6):
                t = singles.tile([128, 2, E], F8, tag=f"w2_{kt}")
                nc.scalar.dma_start(t[:], w2f8_d[kt])
                w2_sb.append(t)

        bq_sb = singles.tile([128, 3], F32, tag="bq")
        bk_sb = singles.tile([128, 3], F32, tag="bk")
        b1_sb = singles.tile([128, 12], F32, tag="b1")
        bo_bc = singles.tile([128, E], F32, tag="bo_bc")
        b2_bc = singles.tile([128, E], F32, tag="b2_bc")
        maskt = singles.tile([128, 128], BF16, tag="maskt")
        ones64 = singles.tile([128, 64], BF16, tag="ones64")

        def load_biases():
            for t, srcd in ((bq_sb, bq_d), (bk_sb, bk_d), (b1_sb, b1_d),
                            (bo_bc, bo_d), (b2_bc, b2_d), (maskt, mask_d),
                            (ones64, ones_d)):
                nc.scalar.dma_start(t[:], srcd[:])

        eps_t = singles.tile([128, 1], F32, tag="eps")
        nc.vector.memset(eps_t[:], EPS)
        warm_act = singles.tile([128, 1], F32, tag="warm_act")
        nc.scalar.activation(warm_act[:], eps_t[:], AF.Exp)

        def layernorm_quad(pairs):
            """For each (src, dst) in pairs (up to 4): dst (bf16) =
            (src - mean) * rsqrt(var + eps) row-wise over 384.
            The rsqrt is exp(-0.5*ln(var+eps)) batched over all tiles so the
            per-op ACT overhead is paid once, and only the exp/ln table set
            is ever touched."""
            n = len(pairs)
            mv_all = p_sm.tile([128, n, 2], F32, tag="bnmv")
            for j, (src, _) in enumerate(pairs):
                st = p_sm.tile([128, 6], F32, tag="bnst")
                nc.vector.bn_stats(st[:], src[:])
                nc.vector.bn_aggr(mv_all[:, j, :], st[:])
            lnv = p_sm.tile([128, n], F32, tag="lnv")
            nc.scalar.activation(lnv[:], mv_all[:, :, 1], AF.Ln, bias=eps_t[:])
            rstd = p_sm.tile([128, n], F32, tag="rstd")
            nc.scalar.activation(rstd[:], lnv[:], AF.Exp, scale=-0.5)
            for j, (src, dst) in enumerate(pairs):
                nc.vector.tensor_scalar(
                    dst[:], src[:], mv_all[:, j, 0:1], rstd[:, j : j + 1],
                    op0=OP.subtract, op1=OP.mult,
                )

        # ---- main loop: 16 pairs of batch items, software-pipelined ----
        # Stage A (load, LN1, transpose, QKV) of pair pp+1 is emitted before
        # stage B (attention, sa, LN2, FFN, out) of pair pp so the PE always
        # has independent matmul work during B's LN2 serial chain (keeps the
        # HAM clock gate warm).

        def stage_a1(pp):
            x_sb = [[None, None], [None, None]]
            h1_sb = [[None, None], [None, None]]
            ln_pairs = []
            for it in range(2):
                i = 2 * pp + it
                for tt in range(2):
                    xt = p_x.tile([128, E], BF16, tag="x")
                    nc.sync.dma_start(
                        xt[:], x_d[i, tt * 128 : (tt + 1) * 128, :]
                    )
                    x_sb[it][tt] = xt
                    h1 = p_h.tile([128, E], BF16, tag="h1")
                    ln_pairs.append((xt, h1))
                    h1_sb[it][tt] = h1
            layernorm_quad(ln_pairs)

            # transpose h1 -> h1T [E, 2*T] (feature-major) via DMA xbar
            h1T = p_h1T.tile([128, 3, 512], BF16, tag="h1T")
            for it in range(2):
                for tt in range(2):
                    c0 = it * 256 + tt * 128
                    nc.sync.dma_start_transpose(
                        out=h1T[:, :, c0 : c0 + 128], in_=h1_sb[it][tt][:]
                    )
            # fp8 copy of feature planes 0,1 for the DoubleRow matmuls
            h1T8 = p_h1T.tile([128, 2, 512], F8, tag="h1T8")
            nc.vector.tensor_copy(h1T8[:], h1T[:, 0:2, :])
            return dict(x_sb=x_sb, h1T=h1T, h1T8=h1T8)

        def stage_a2(pp, st):
            h1T, h1T8 = st["h1T"], st["h1T8"]
            # qT, kT projections (feature-major): qT[f, t] over both items.
            # Weights are prescaled x64 on the host (fp8 normal range); the
            # eviction's free activation scale undoes it exactly.
            qT = p_qk.tile([128, 3, 512], BF16, tag="qT")
            kT = p_qk.tile([128, 3, 512], BF16, tag="kT")
            for dst, w8, wb, b_sb in (
                (qT, w8_sb["q"], wb_sb["q"], bq_sb),
                (kT, w8_sb["k"], wb_sb["k"], bk_sb),
            ):
                for ft in range(3):
                    ps = pp_big.tile([128, 512], F32, tag="ps_big")
                    nc.tensor.matmul(
                        ps[:], wb[:, ft * 128 : (ft + 1) * 128], h1T[:, 2, :],
                        start=True, stop=False,
                    )
                    nc.tensor.matmul(
                        ps[:], w8[:, :, ft * 128 : (ft + 1) * 128], h1T8[:],
                        start=False, stop=True, perf_mode=DR,
                    )
                    nc.scalar.activation(
                        dst[:, ft, :], ps[:], AF.Identity,
                        bias=b_sb[:, ft : ft + 1], scale=1.0 / 64,
                    )

            # v (token-major): v[t, f] per item
            v_sb = [None, None]
            for it in range(2):
                vt = p_h.tile([128, 2, E], BF16, tag="v")
                for tt in range(2):
                    c0 = it * 256 + tt * 128
                    ps = pp_v.tile([128, E], F32, tag="ps_v")
                    nc.tensor.matmul(
                        ps[:], h1T[:, 2, c0 : c0 + 128], wb_sb["v"][:],
                        start=True, stop=False,
                    )
                    nc.tensor.matmul(
                        ps[:], h1T8[:, :, c0 : c0 + 128], w8_sb["v"][:],
                        start=False, stop=True, perf_mode=DR,
                    )
                    nc.scalar.activation(vt[:, tt, :], ps[:], AF.Identity, scale=1.0 / 64)
                v_sb[it] = vt
            st["qT"], st["kT"], st["v_sb"] = qT, kT, v_sb

        def stage_b1(pp, st):
            qT, kT, v_sb = st["qT"], st["kT"], st["v_sb"]
            # residual bases with biases folded in (on the idle GPSIMD):
            # xbo = x + bo' (feeds y), xb2 = x + b2 (feeds final out)
            xbo = [[None, None], [None, None]]
            xb2 = [[None, None], [None, None]]
            for it in range(2):
                for tt in range(2):
                    a = p_xb.tile([128, E], F32, tag="xbo")
                    nc.gpsimd.tensor_add(a[:], st["x_sb"][it][tt][:], bo_bc[:])
                    xbo[it][tt] = a
                    b = p_xb.tile([128, E], F32, tag="xb2")
                    nc.gpsimd.tensor_add(b[:], st["x_sb"][it][tt][:], b2_bc[:])
                    xb2[it][tt] = b
            st["xbo"], st["xb2"] = xbo, xb2
            # attention, transposed layout: scoresT[k, q] per (item, head)
            attnT = p_T.tile([128, 3, 512], BF16, tag="attnT")
            for it in range(2):
                c0 = it * 256
                expA = p_e.tile([128, 6, 384], BF16, tag="expA")
                for h in range(H):
                    p3 = h // 2
                    r0 = 64 * (h % 2)
                    qh = qT[r0 : r0 + 64, p3, c0 : c0 + 256]
                    ps_s = pp_s.tile([128, 384], F32, tag="ps_s")
                    # k-tile 0: all q columns
                    nc.tensor.matmul(
                        ps_s[:, 0:256],
                        kT[r0 : r0 + 64, p3, c0 : c0 + 128],
                        qh,
                        start=True,
                        stop=True,
                        skip_group_check=True,
                    )
                    # k-tile 1: only q columns 128:256 (rest fully masked)
                    nc.tensor.matmul(
                        ps_s[:, 256:384],
                        kT[r0 : r0 + 64, p3, c0 + 128 : c0 + 256],
                        qT[r0 : r0 + 64, p3, c0 + 128 : c0 + 256],
                        start=True,
                        stop=True,
                        skip_group_check=True,
                    )
                    # one fused exp over both score blocks
                    nc.scalar.activation(expA[:, h, :], ps_s[:], AF.Exp)
                    if h % 2 == 1:
                        # causal masks for this head pair: diag block of
                        # k-tile 0 (q 0:128) and the k-tile 1 block
                        # (on GpSimd: frees DVE for the LN2/eviction chains)
                        hp = h // 2
                        nc.vector.tensor_tensor(
                            expA[:, 2 * hp : 2 * hp + 2, 0:128],
                            expA[:, 2 * hp : 2 * hp + 2, 0:128],
                            maskt[:, None, :].to_broadcast((128, 2, 128)),
                            op=OP.mult,
                        )
                        nc.vector.tensor_tensor(
                            expA[:, 2 * hp : 2 * hp + 2, 256:384],
                            expA[:, 2 * hp : 2 * hp + 2, 256:384],
                            maskt[:, None, :].to_broadcast((128, 2, 128)),
                            op=OP.mult,
                        )

                # head pairs (2h, 2h+1) share a partition tile: pack their
                # denominator and output matmuls into one [128, 256] psum via
                # col tile_position, halving the DVE normalize work.
                for hp in range(3):
                    ps_d = pp_do.tile([128, 256], F32, tag="ps_do")
                    ps_o = pp_do.tile([128, 256], F32, tag="ps_do")
                    for sub in range(2):
                        h = 2 * hp + sub
                        r0 = 64 * sub
                        tp = (0, r0)
                        e0 = expA[:, h, 0:256]
                        e1 = expA[:, h, 256:384]
                        nc.tensor.matmul(
                            ps_d[r0 : r0 + 64, :], ones64[:], e0,
                            start=True, stop=False,
                            skip_group_check=True, tile_position=tp,
                        )
                        nc.tensor.matmul(
                            ps_d[r0 : r0 + 64, 128:256], ones64[:], e1,
                            start=False, stop=True,
                            skip_group_check=True, tile_position=tp,
                        )
                        nc.tensor.matmul(
                            ps_o[r0 : r0 + 64, :],
                            v_sb[it][:, 0, 64 * h : 64 * h + 64], e0,
                            start=True, stop=False,
                            skip_group_check=True, tile_position=tp,
                        )
                        nc.tensor.matmul(
                            ps_o[r0 : r0 + 64, 128:256],
                            v_sb[it][:, 1, 64 * h : 64 * h + 64], e1,
                            start=False, stop=True,
                            skip_group_check=True, tile_position=tp,
                        )
                    rec = p_rec.tile([128, 256], F32, tag="rec")
                    nc.vector.reciprocal_approx_fast(rec[:], ps_d[:])
                    nc.vector.tensor_tensor(
                        attnT[:, hp, c0 : c0 + 256], ps_o[:], rec[:], op=OP.mult,
                    )

            st["attnT"] = attnT

        def warm(n=1):
            for _ in range(n):
                pw = pp_v.tile([64, 384], F32, tag="ps_v")
                nc.tensor.matmul(
                    pw[:], ones64[:], wo_sb[0][:], start=True, stop=True,
                    skip_group_check=True,
                )

        def stage_b2a(pp, st):
            attnT = st["attnT"]
            # sa (token-major) + residual + bo, then LN2
            h2_sb = [[None, None], [None, None]]
            ln2_pairs = []
            for it in range(2):
                for tt in range(2):
                    ps = pp_v.tile([128, E], F32, tag="ps_v")
                    for kt in range(3):
                        nc.tensor.matmul(
                            ps[:],
                            attnT[:, kt, it * 256 + tt * 128 : it * 256 + tt * 128 + 128],
                            wo_sb[kt][:],
                            start=(kt == 0),
                            stop=(kt == 2),
                        )
                    y = p_y.tile([128, E], F32, tag="y")
                    nc.vector.tensor_add(y[:], ps[:], st["xbo"][it][tt][:])
                    h2 = p_h.tile([128, E], BF16, tag="h2")
                    ln2_pairs.append((y, h2))
                    h2_sb[it][tt] = h2

            layernorm_quad(ln2_pairs)
            warm(3)

            # transpose h2 -> h2T via DMA xbar (PE transpose-mode does not
            # count as HAM activity and was re-throttling the PE clock)
            h2T = p_T.tile([128, 3, 512], BF16, tag="h2T")
            h2T8 = p_T.tile([128, 2, 512], F8, tag="h2T8")
            for it in range(2):
                for tt in range(2):
                    c0 = it * 256 + tt * 128
                    nc.sync.dma_start_transpose(
                        out=h2T[:, :, c0 : c0 + 128], in_=h2_sb[it][tt][:]
                    )
            nc.vector.tensor_copy(h2T8[:], h2T[:, 0:2, :])
            st["h2T"], st["h2T8"] = h2T, h2T8

        def stage_b2b(pp, st):
            h2T, h2T8 = st["h2T"], st["h2T8"]
            # FFN: reluT[c, t] = relu(W1'^T @ h2T + b1'), feature-major.
            # fp8 DoubleRow over features 0-255 + bf16 tail; w1/w2 quantized
            # at natural scale so both eviction engines need no rescale.
            relu_sb = p_relu.tile([128, 6, 2, 512], F8, tag="relu")
            for mt in range(12):
                ps = pp_big.tile([128, 512], F32, tag="ps_big")
                for it in range(2):
                    c0 = it * 256
                    nc.tensor.matmul(
                        ps[:, c0 : c0 + 256],
                        wb_sb["1"][:, mt * 128 : (mt + 1) * 128],
                        h2T[:, 2, c0 : c0 + 256],
                        start=True, stop=False,
                        skip_group_check=True,
                    )
                    nc.tensor.matmul(
                        ps[:, c0 : c0 + 256],
                        w8_sb["1"][:, :, mt * 128 : (mt + 1) * 128],
                        h2T8[:, :, c0 : c0 + 256],
                        start=False, stop=True, perf_mode=DR,
                        skip_group_check=True,
                    )
                dst = relu_sb[:, mt // 2, mt % 2, :]
                if mt % 2 == 0:
                    nc.scalar.activation(
                        dst, ps[:], AF.Relu, bias=b1_sb[:, mt : mt + 1]
                    )
                else:
                    # split evictions across ScalarE/DVE: the serial relu
                    # chain paces FFN2 otherwise
                    nc.vector.tensor_scalar(
                        dst, ps[:], b1_sb[:, mt : mt + 1], 0.0,
                        op0=OP.add, op1=OP.max,
                    )

            # ff (token-major) + residual to x + b2, DMA out
            for it in range(2):
                i = 2 * pp + it
                for tt in range(2):
                    c0 = it * 256 + tt * 128
                    ps = pp_v.tile([128, E], F32, tag="ps_v")
                    for kt in range(6):
                        nc.tensor.matmul(
                            ps[:],
                            relu_sb[:, kt, :, c0 : c0 + 128],
                            w2_sb[kt][:],
                            start=(kt == 0),
                            stop=(kt == 5),
                            perf_mode=DR,
                        )
                    ot = p_y.tile([128, E], BF16, tag="ot")
                    nc.vector.tensor_add(ot[:], ps[:], st["xb2"][it][tt][:])
                    nc.sync.dma_start(out_d[i, tt * 128 : (tt + 1) * 128, :], ot[:])

        n_pairs = IPC // 2
        sts = {}
        sts[0] = stage_a1(0)
        load_weights()
        load_biases()
        for k in range(1, min(3, n_pairs)):
            sts[k] = stage_a1(k)
        for k in range(min(2, n_pairs)):
            stage_a2(k, sts[k])
        stage_b1(0, sts[0])
        for pp in range(n_pairs):
            if pp + 3 < n_pairs:
                sts[pp + 3] = stage_a1(pp + 3)
            if pp + 2 < n_pairs:
                stage_a2(pp + 2, sts[pp + 2])
            if pp + 1 < n_pairs:
                stage_b1(pp + 1, sts[pp + 1])
            stage_b2a(pp, sts[pp])
            stage_b2b(pp, sts.pop(pp))

    nc.compile()
    return nc


def _prep_inputs(inputs):
    f = lambda v: np.asarray(v, dtype=np.float32)
    x = f(inputs["x"])
    Wq, Wk, Wv, Wo = f(inputs["Wq"]), f(inputs["Wk"]), f(inputs["Wv"]), f(inputs["Wo"])
    bo = f(inputs["bo"])
    W1, b1, W2, b2 = f(inputs["W1"]), f(inputs["b1"]), f(inputs["W2"]), f(inputs["b2"])
    g1, be1 = f(inputs["g1"]), f(inputs["be1"])
    g2, be2 = f(inputs["g2"]), f(inputs["be2"])

    scale = HS ** -0.5
    wq = (g1[:, None] * Wq) * scale
    bq = ((be1 @ Wq) * scale).astype(np.float32)
    wk = g1[:, None] * Wk
    bk = (be1 @ Wk).astype(np.float32)
    wv = g1[:, None] * Wv
    bv = be1 @ Wv
    # bv folded through Wo (softmax rows sum to 1): bo' = bv @ Wo + bo
    bo_p = (bv @ Wo + bo).astype(np.float32)
    w1 = g2[:, None] * W1
    b1_p = (be2 @ W1 + b1).astype(np.float32)

    def q8(a):
        return np.clip(a, -240.0, 240.0).astype(F8NP)

    def pack2(a):  # [256, M] -> [128, 2, M] DoubleRow planes
        return np.ascontiguousarray(a.reshape(2, 128, -1).transpose(1, 0, 2))

    col128 = lambda v: np.ascontiguousarray(v.reshape(-1, 128).T.astype(np.float32))
    row128 = lambda v: np.ascontiguousarray(
        np.broadcast_to(v.astype(np.float32), (128, v.shape[0]))
    )
    common = {
        # q/k/v prescaled x64 (fp8 normal range); undone by eviction scale
        "wq8": q8(pack2(wq[:256] * 64)), "wqb": (wq[256:] * 64).astype(BF),
        "wk8": q8(pack2(wk[:256] * 64)), "wkb": (wk[256:] * 64).astype(BF),
        "wv8": q8(pack2(wv[:256] * 64)), "wvb": (wv[256:] * 64).astype(BF),
        "wo": Wo.astype(BF),
        "w1f8": q8(pack2(w1[:256])), "w1b": w1[256:].astype(BF),
        "w2f8": q8(
            np.ascontiguousarray(W2.reshape(6, 2, 128, E).transpose(0, 2, 1, 3))
        ),
        "bq": col128(bq), "bk": col128(bk), "b1": col128(b1_p),
        "bo": row128(bo_p), "b2": row128(b2.astype(np.float32)),
        "maskt": np.triu(np.ones((128, 128), np.float32)).astype(BF),
        "ones64": np.ones((128, 64), np.float32).astype(BF),
    }
    in_maps = []
    for c in range(N_CORES):
        m = dict(common)
        m["x"] = np.ascontiguousarray(x[c * IPC : (c + 1) * IPC]).astype(BF)
        in_maps.append(m)
    return in_maps


def kernel(**inputs):
    if "nc" not in _CACHE:
        _CACHE["nc"] = _build()
    nc = _CACHE["nc"]
    in_maps = _prep_inputs(inputs)
    res = run_bass_kernel_spmd(nc, in_maps, core_ids=list(range(N_CORES)))
    _CACHE["last_result"] = res
    return np.concatenate([r["out"] for r in res.results], axis=0).astype(np.float32)



# revision 34
# speedup vs baseline: 1.1631x; 1.0037x over previous
"""Trainium2 Bass kernel for a dense transformer block.

Computes: ffwd(ln2(sa(ln1(x)) + x)) + x  (residual 2 connects to x)
with causal self-attention (6 heads, head_size 64), seq len 256, n_embed 384.

Sharding: data-parallel over batch (B=256) -> 32 items per NeuronCore,
weights replicated. All LN gains/biases and the softmax scale are folded
into the weight matrices on the host; matmul inputs are bf16 with fp32
PSUM accumulation; the LN/residual spine stays fp32.
"""

import sys
from contextlib import ExitStack

sys.path.insert(0, "/opt/trn_rl_repo")

import numpy as np
import ml_dtypes

import concourse.bass as bass
import concourse.tile as tile
from concourse import bacc, mybir
from concourse.bass_utils import run_bass_kernel_spmd

B, T, E, H, HS = 256, 256, 384, 6, 64
FF = 4 * E  # 1536
N_CORES = 8
IPC = B // N_CORES  # items per core
EPS = 1e-5

BF16 = mybir.dt.bfloat16
F32 = mybir.dt.float32
F8 = mybir.dt.float8e4
AF = mybir.ActivationFunctionType
OP = mybir.AluOpType
DR = mybir.MatmulPerfMode.DoubleRow
BF = ml_dtypes.bfloat16
F8NP = ml_dtypes.float8_e4m3

_CACHE = {}


def _setup_act_tables():
    """Force a single ACT table set (natural_log_exp_and_others) so walrus
    never thrashes between the exp and ln sets: we only use ln/exp/relu/
    copy/identity, which all live in that one set."""
    import os, json, tempfile

    if "BASS_ACT_ROOT_JSON_PATH" in os.environ:
        return
    from neuronxcc.driver.Job import Job
    from neuronxcc.driver.jobs.support.FindActInfo import findActInfoFile

    src = findActInfoFile(Job.getPackageDir(), "gen3")
    d = json.load(open(src))
    keep = [s for s in d["act_func_sets"] if s["name"] == "natural_log_exp_and_others"]
    assert keep, "natural_log_exp_and_others set not found"
    d["act_func_sets"] = keep
    dst_dir = tempfile.mkdtemp(prefix="act_custom_")
    srcdir = os.path.dirname(src)
    for key in d["pwp_file_keys"]:
        fn = keep[0][key]
        os.symlink(os.path.join(srcdir, fn), os.path.join(dst_dir, fn))
    dst = os.path.join(dst_dir, "act_info.json")
    with open(dst, "w") as f:
        json.dump(d, f)
    os.environ["BASS_ACT_ROOT_JSON_PATH"] = dst

    # Bacc's insert_act_table_loads must agree with walrus on set ids:
    # filter its table view to the same single set (id 0).
    import concourse.hw_specs as hw_specs
    import concourse.bacc as bacc_mod

    orig = hw_specs.get_activation_tables

    def filtered(arch):
        t = orig(arch)
        return {"natural_log_exp_and_others": t["natural_log_exp_and_others"]}

    hw_specs.get_activation_tables = filtered
    bacc_mod.get_activation_tables = filtered


def _build():
    _setup_act_tables()
    nc = bacc.Bacc("TRN2", target_bir_lowering=False, debug=False)

    x_d = nc.dram_tensor("x", [IPC, T, E], BF16, kind="ExternalInput").ap()
    # q/k/v/ffn1 weights: features 0-255 as fp8 DoubleRow planes, 256-383 bf16
    wq8_d = nc.dram_tensor("wq8", [128, 2, E], F8, kind="ExternalInput").ap()
    wk8_d = nc.dram_tensor("wk8", [128, 2, E], F8, kind="ExternalInput").ap()
    wv8_d = nc.dram_tensor("wv8", [128, 2, E], F8, kind="ExternalInput").ap()
    wqb_d = nc.dram_tensor("wqb", [128, E], BF16, kind="ExternalInput").ap()
    wkb_d = nc.dram_tensor("wkb", [128, E], BF16, kind="ExternalInput").ap()
    wvb_d = nc.dram_tensor("wvb", [128, E], BF16, kind="ExternalInput").ap()
    wo_d = nc.dram_tensor("wo", [E, E], BF16, kind="ExternalInput").ap()
    w1f8_d = nc.dram_tensor("w1f8", [128, 2, FF], F8, kind="ExternalInput").ap()
    w1b_d = nc.dram_tensor("w1b", [128, FF], BF16, kind="ExternalInput").ap()
    w2f8_d = nc.dram_tensor("w2f8", [6, 128, 2, E], F8, kind="ExternalInput").ap()
    # biases pre-laid-out on the host: [128, n] column tiles / [128, E] rows
    bq_d = nc.dram_tensor("bq", [128, 3], F32, kind="ExternalInput").ap()
    bk_d = nc.dram_tensor("bk", [128, 3], F32, kind="ExternalInput").ap()
    b1_d = nc.dram_tensor("b1", [128, 12], F32, kind="ExternalInput").ap()
    bo_d = nc.dram_tensor("bo", [128, E], F32, kind="ExternalInput").ap()
    b2_d = nc.dram_tensor("b2", [128, E], F32, kind="ExternalInput").ap()
    mask_d = nc.dram_tensor("maskt", [128, 128], BF16, kind="ExternalInput").ap()
    ones_d = nc.dram_tensor("ones64", [128, 64], BF16, kind="ExternalInput").ap()
    out_d = nc.dram_tensor("out", [IPC, T, E], BF16, kind="ExternalOutput").ap()

    with tile.TileContext(nc) as tc, ExitStack() as ctx:
        singles = ctx.enter_context(tc.tile_pool(name="singles", bufs=1))
        p_x = ctx.enter_context(tc.tile_pool(name="p_x", bufs=16))
        p_h = ctx.enter_context(tc.tile_pool(name="p_h", bufs=8))
        p_T = ctx.enter_context(tc.tile_pool(name="p_T", bufs=2))
        p_h1T = ctx.enter_context(tc.tile_pool(name="p_h1T", bufs=4))
        p_relu = ctx.enter_context(tc.tile_pool(name="p_relu", bufs=2))
        p_xb = ctx.enter_context(tc.tile_pool(name="p_xb", bufs=8))
        p_rec = ctx.enter_context(tc.tile_pool(name="p_rec", bufs=4))
        p_qk = ctx.enter_context(tc.tile_pool(name="p_qk", bufs=3))
        p_e = ctx.enter_context(tc.tile_pool(name="p_e", bufs=2))
        p_sm = ctx.enter_context(tc.tile_pool(name="p_sm", bufs=16))
        p_y = ctx.enter_context(tc.tile_pool(name="p_y", bufs=6))

        pp_big = ctx.enter_context(tc.tile_pool(name="pp_big", bufs=2, space="PSUM"))
        pp_v = ctx.enter_context(tc.tile_pool(name="pp_v", bufs=2, space="PSUM"))
        pp_s = ctx.enter_context(tc.tile_pool(name="pp_s", bufs=2, space="PSUM"))
        pp_do = ctx.enter_context(tc.tile_pool(name="pp_do", bufs=2, space="PSUM"))

        # ---- constants / weights (emission deferred via load_weights so the
        # first pair's x DMAs + LN1 go out ahead of the bulk weight traffic) ----
        w8_sb = {}
        wb_sb = {}
        wo_sb = []
        w2_sb = []

        def load_weights():
            # qkv first (stage_a2(0) needs them soonest), then wo/w1/w2
            for nm, src8, srcb in (
                ("q", wq8_d, wqb_d), ("k", wk8_d, wkb_d), ("v", wv8_d, wvb_d)
            ):
                t8 = singles.tile([128, 2, E], F8, tag=f"w8_{nm}")
                nc.scalar.dma_start(t8[:], src8[:])
                w8_sb[nm] = t8
                tb = singles.tile([128, E], BF16, tag=f"wb_{nm}")
                nc.scalar.dma_start(tb[:], srcb[:])
                wb_sb[nm] = tb
            for kt in range(3):
                t = singles.tile([128, E], BF16, tag=f"wo_{kt}")
                nc.scalar.dma_start(t[:], wo_d[kt * 128 : (kt + 1) * 128, :])
                wo_sb.append(t)
            t8 = singles.tile([128, 2, FF], F8, tag="w8_1")
            nc.scalar.dma_start(t8[:], w1f8_d[:])
            w8_sb["1"] = t8
            tb = singles.tile([128, FF], BF16, tag="wb_1")
            nc.scalar.dma_start(tb[:], w1b_d[:])
            wb_sb["1"] = tb
            for kt in range(6):
                t = singles.tile([128, 2, E], F8, tag=f"w2_{kt}")
                nc.scalar.dma_start(t[:], w2f8_d[kt])
                w2_sb.append(t)

        bq_sb = singles.tile([128, 3], F32, tag="bq")
        bk_sb = singles.tile([128, 3], F32, tag="bk")
        b1_sb = singles.tile([128, 12], F32, tag="b1")
        bo_bc = singles.tile([128, E], F32, tag="bo_bc")
        b2_bc = singles.tile([128, E], F32, tag="b2_bc")
        maskt = singles.tile([128, 128], BF16, tag="maskt")
        ones64 = singles.tile([128, 64], BF16, tag="ones64")

        def load_biases():
            for t, srcd in ((bq_sb, bq_d), (bk_sb, bk_d), (b1_sb, b1_d),
                            (bo_bc, bo_d), (b2_bc, b2_d), (maskt, mask_d),
                            (ones64, ones_d)):
                nc.scalar.dma_start(t[:], srcd[:])

        eps_t = singles.tile([128, 1], F32, tag="eps")
        nc.vector.memset(eps_t[:], EPS)
        warm_act = singles.tile([128, 1], F32, tag="warm_act")
        nc.scalar.activation(warm_act[:], eps_t[:], AF.Exp)

        def layernorm_quad(pairs):
            """For each (src, dst) in pairs (up to 4): dst (bf16) =
            (src - mean) * rsqrt(var + eps) row-wise over 384.
            The rsqrt is exp(-0.5*ln(var+eps)) batched over all tiles so the
            per-op ACT overhead is paid once, and only the exp/ln table set
            is ever touched."""
            n = len(pairs)
            mv_all = p_sm.tile([128, n, 2], F32, tag="bnmv")
            for j, (src, _) in enumerate(pairs):
                st = p_sm.tile([128, 6], F32, tag="bnst")
                nc.vector.bn_stats(st[:], src[:])
                nc.vector.bn_aggr(mv_all[:, j, :], st[:])
            lnv = p_sm.tile([128, n], F32, tag="lnv")
            nc.scalar.activation(lnv[:], mv_all[:, :, 1], AF.Ln, bias=eps_t[:])
            rstd = p_sm.tile([128, n], F32, tag="rstd")
            nc.scalar.activation(rstd[:], lnv[:], AF.Exp, scale=-0.5)
            for j, (src, dst) in enumerate(pairs):
                nc.vector.tensor_scalar(
                    dst[:], src[:], mv_all[:, j, 0:1], rstd[:, j : j + 1],
                    op0=OP.subtract, op1=OP.mult,
                )

        # ---- main loop: 16 pairs of batch items, software-pipelined ----
        # Stage A (load, LN1, transpose, QKV) of pair pp+1 is emitted before
        # stage B (attention, sa, LN2, FFN, out) of pair pp so the PE always
        # has independent matmul work during B's LN2 serial chain (keeps the
        # HAM clock gate warm).

        def stage_a1(pp):
            x_sb = [[None, None], [None, None]]
            h1_sb = [[None, None], [None, None]]
            ln_pairs = []
            for it in range(2):
                i = 2 * pp + it
                for tt in range(2):
                    xt = p_x.tile([128, E], BF16, tag="x")
                    nc.sync.dma_start(
                        xt[:], x_d[i, tt * 128 : (tt + 1) * 128, :]
                    )
                    x_sb[it][tt] = xt
                    h1 = p_h.tile([128, E], BF16, tag="h1")
                    ln_pairs.append((xt, h1))
                    h1_sb[it][tt] = h1
            layernorm_quad(ln_pairs)

            # transpose h1 -> h1T [E, 2*T] (feature-major) via DMA xbar
            h1T = p_h1T.tile([128, 3, 512], BF16, tag="h1T")
            for it in range(2):
                for tt in range(2):
                    c0 = it * 256 + tt * 128
                    nc.sync.dma_start_transpose(
                        out=h1T[:, :, c0 : c0 + 128], in_=h1_sb[it][tt][:]
                    )
            # fp8 copy of feature planes 0,1 for the DoubleRow matmuls
            h1T8 = p_h1T.tile([128, 2, 512], F8, tag="h1T8")
            nc.vector.tensor_copy(h1T8[:], h1T[:, 0:2, :])
            return dict(x_sb=x_sb, h1T=h1T, h1T8=h1T8)

        def stage_a2(pp, st):
            h1T, h1T8 = st["h1T"], st["h1T8"]
            # qT, kT projections (feature-major): qT[f, t] over both items.
            # Weights are prescaled x64 on the host (fp8 normal range); the
            # eviction's free activation scale undoes it exactly.
            qT = p_qk.tile([128, 3, 512], BF16, tag="qT")
            kT = p_qk.tile([128, 3, 512], BF16, tag="kT")
            for dst, w8, wb, b_sb in (
                (qT, w8_sb["q"], wb_sb["q"], bq_sb),
                (kT, w8_sb["k"], wb_sb["k"], bk_sb),
            ):
                for ft in range(3):
                    ps = pp_big.tile([128, 512], F32, tag="ps_big")
                    nc.tensor.matmul(
                        ps[:], wb[:, ft * 128 : (ft + 1) * 128], h1T[:, 2, :],
                        start=True, stop=False,
                    )
                    nc.tensor.matmul(
                        ps[:], w8[:, :, ft * 128 : (ft + 1) * 128], h1T8[:],
                        start=False, stop=True, perf_mode=DR,
                    )
                    nc.scalar.activation(
                        dst[:, ft, :], ps[:], AF.Identity,
                        bias=b_sb[:, ft : ft + 1], scale=1.0 / 64,
                    )

            # v (token-major): v[t, f] per item
            v_sb = [None, None]
            for it in range(2):
                vt = p_h.tile([128, 2, E], BF16, tag="v")
                for tt in range(2):
                    c0 = it * 256 + tt * 128
                    ps = pp_v.tile([128, E], F32, tag="ps_v")
                    nc.tensor.matmul(
                        ps[:], h1T[:, 2, c0 : c0 + 128], wb_sb["v"][:],
                        start=True, stop=False,
                    )
                    nc.tensor.matmul(
                        ps[:], h1T8[:, :, c0 : c0 + 128], w8_sb["v"][:],
                        start=False, stop=True, perf_mode=DR,
                    )
                    nc.scalar.activation(vt[:, tt, :], ps[:], AF.Identity, scale=1.0 / 64)
                v_sb[it] = vt
            st["qT"], st["kT"], st["v_sb"] = qT, kT, v_sb

        def stage_b1(pp, st):
            qT, kT, v_sb = st["qT"], st["kT"], st["v_sb"]
            # residual bases with biases folded in (on the idle GPSIMD):
            # xbo = x + bo' (feeds y), xb2 = x + b2 (feeds final out)
            xbo = [[None, None], [None, None]]
            xb2 = [[None, None], [None, None]]
            for it in range(2):
                for tt in range(2):
                    a = p_xb.tile([128, E], F32, tag="xbo")
                    nc.gpsimd.tensor_add(a[:], st["x_sb"][it][tt][:], bo_bc[:])
                    xbo[it][tt] = a
                    b = p_xb.tile([128, E], F32, tag="xb2")
                    nc.gpsimd.tensor_add(b[:], st["x_sb"][it][tt][:], b2_bc[:])
                    xb2[it][tt] = b
            st["xbo"], st["xb2"] = xbo, xb2
            # attention, transposed layout: scoresT[k, q] per (item, head)
            attnT = p_T.tile([128, 3, 512], BF16, tag="attnT")
            for it in range(2):
                c0 = it * 256
                expA = p_e.tile([128, 6, 384], BF16, tag="expA")
                for h in range(H):
                    p3 = h // 2
                    r0 = 64 * (h % 2)
                    qh = qT[r0 : r0 + 64, p3, c0 : c0 + 256]
                    ps_s = pp_s.tile([128, 384], F32, tag="ps_s")
                    # k-tile 0: all q columns
                    nc.tensor.matmul(
                        ps_s[:, 0:256],
                        kT[r0 : r0 + 64, p3, c0 : c0 + 128],
                        qh,
                        start=True,
                        stop=True,
                        skip_group_check=True,
                    )
                    # k-tile 1: only q columns 128:256 (rest fully masked)
                    nc.tensor.matmul(
                        ps_s[:, 256:384],
                        kT[r0 : r0 + 64, p3, c0 + 128 : c0 + 256],
                        qT[r0 : r0 + 64, p3, c0 + 128 : c0 + 256],
                        start=True,
                        stop=True,
                        skip_group_check=True,
                    )
                    # one fused exp over both score blocks
                    nc.scalar.activation(expA[:, h, :], ps_s[:], AF.Exp)
                    if h % 2 == 1:
                        # causal masks for this head pair: diag block of
                        # k-tile 0 (q 0:128) and the k-tile 1 block
                        # (on GpSimd: frees DVE for the LN2/eviction chains)
                        hp = h // 2
                        nc.vector.tensor_tensor(
                            expA[:, 2 * hp : 2 * hp + 2, 0:128],
                            expA[:, 2 * hp : 2 * hp + 2, 0:128],
                            maskt[:, None, :].to_broadcast((128, 2, 128)),
                            op=OP.mult,
                        )
                        nc.vector.tensor_tensor(
                            expA[:, 2 * hp : 2 * hp + 2, 256:384],
                            expA[:, 2 * hp : 2 * hp + 2, 256:384],
                            maskt[:, None, :].to_broadcast((128, 2, 128)),
                            op=OP.mult,
                        )

                # head pairs (2h, 2h+1) share a partition tile: pack their
                # denominator and output matmuls into one [128, 256] psum via
                # col tile_position, halving the DVE normalize work.
                for hp in range(3):
                    ps_d = pp_do.tile([128, 256], F32, tag="ps_do")
                    ps_o = pp_do.tile([128, 256], F32, tag="ps_do")
                    for sub in range(2):
                        h = 2 * hp + sub
                        r0 = 64 * sub
                        tp = (0, r0)
                        e0 = expA[:, h, 0:256]
                        e1 = expA[:, h, 256:384]
                        nc.tensor.matmul(
                            ps_d[r0 : r0 + 64, :], ones64[:], e0,
                            start=True, stop=False,
                            skip_group_check=True, tile_position=tp,
                        )
                        nc.tensor.matmul(
                            ps_d[r0 : r0 + 64, 128:256], ones64[:], e1,
                            start=False, stop=True,
                            skip_group_check=True, tile_position=tp,
                        )
                        nc.tensor.matmul(
                            ps_o[r0 : r0 + 64, :],
                            v_sb[it][:, 0, 64 * h : 64 * h + 64], e0,
                            start=True, stop=False,
                            skip_group_check=True, tile_position=tp,
                        )
                        nc.tensor.matmul(
                            ps_o[r0 : r0 + 64, 128:256],
                            v_sb[it][:, 1, 64 * h : 64 * h + 64], e1,
                            start=False, stop=True,
                            skip_group_check=True, tile_position=tp,
                        )
                    rec = p_rec.tile([128, 256], F32, tag="rec")
                    nc.vector.reciprocal_approx_fast(rec[:], ps_d[:])
                    nc.vector.tensor_tensor(
                        attnT[:, hp, c0 : c0 + 256], ps_o[:], rec[:], op=OP.mult,
                    )

            st["attnT"] = attnT

        def warm(n=1):
            for _ in range(n):
                pw = pp_v.tile([64, 384], F32, tag="ps_v")
                nc.tensor.matmul(
                    pw[:], ones64[:], wo_sb[0][:], start=True, stop=True,
                    skip_group_check=True,
                )

        def stage_b2a(pp, st):
            attnT = st["attnT"]
            # sa (token-major) + residual + bo, then LN2
            h2_sb = [[None, None], [None, None]]
            ln2_pairs = []
            for it in range(2):
                for tt in range(2):
                    ps = pp_v.tile([128, E], F32, tag="ps_v")
                    for kt in range(3):
                        nc.tensor.matmul(
                            ps[:],
                            attnT[:, kt, it * 256 + tt * 128 : it * 256 + tt * 128 + 128],
                            wo_sb[kt][:],
                            start=(kt == 0),
                            stop=(kt == 2),
                        )
                    y = p_y.tile([128, E], F32, tag="y")
                    nc.vector.tensor_add(y[:], ps[:], st["xbo"][it][tt][:])
                    h2 = p_h.tile([128, E], BF16, tag="h2")
                    ln2_pairs.append((y, h2))
                    h2_sb[it][tt] = h2

            layernorm_quad(ln2_pairs)
            warm(3)

            # transpose h2 -> h2T via DMA xbar (PE transpose-mode does not
            # count as HAM activity and was re-throttling the PE clock)
            h2T = p_T.tile([128, 3, 512], BF16, tag="h2T")
            h2T8 = p_T.tile([128, 2, 512], F8, tag="h2T8")
            for it in range(2):
                for tt in range(2):
                    c0 = it * 256 + tt * 128
                    nc.sync.dma_start_transpose(
                        out=h2T[:, :, c0 : c0 + 128], in_=h2_sb[it][tt][:]
                    )
            nc.vector.tensor_copy(h2T8[:], h2T[:, 0:2, :])
            st["h2T"], st["h2T8"] = h2T, h2T8

        def stage_b2b(pp, st):
            h2T, h2T8 = st["h2T"], st["h2T8"]
            # FFN: reluT[c, t] = relu(W1'^T @ h2T + b1'), feature-major.
            # fp8 DoubleRow over features 0-255 + bf16 tail; w1/w2 quantized
            # at natural scale so both eviction engines need no rescale.
            relu_sb = p_relu.tile([128, 6, 2, 512], F8, tag="relu")
            for mt in range(12):
                ps = pp_big.tile([128, 512], F32, tag="ps_big")
                for it in range(2):
                    c0 = it * 256
                    nc.tensor.matmul(
                        ps[:, c0 : c0 + 256],
                        wb_sb["1"][:, mt * 128 : (mt + 1) * 128],
                        h2T[:, 2, c0 : c0 + 256],
                        start=True, stop=False,
                        skip_group_check=True,
                    )
                    nc.tensor.matmul(
                        ps[:, c0 : c0 + 256],
                        w8_sb["1"][:, :, mt * 128 : (mt + 1) * 128],
                        h2T8[:, :, c0 : c0 + 256],
                        start=False, stop=True, perf_mode=DR,
                        skip_group_check=True,
                    )
                dst = relu_sb[:, mt // 2, mt % 2, :]
                if mt % 2 == 0:
                    nc.scalar.activation(
                        dst, ps[:], AF.Relu, bias=b1_sb[:, mt : mt + 1]
                    )
                else:
                    # split evictions across ScalarE/DVE: the serial relu
                    # chain paces FFN2 otherwise
                    nc.vector.tensor_scalar(
                        dst, ps[:], b1_sb[:, mt : mt + 1], 0.0,
                        op0=OP.add, op1=OP.max,
                    )

            # ff (token-major) + residual to x + b2, DMA out
            for it in range(2):
                i = 2 * pp + it
                for tt in range(2):
                    c0 = it * 256 + tt * 128
                    ps = pp_v.tile([128, E], F32, tag="ps_v")
                    for kt in range(6):
                        nc.tensor.matmul(
                            ps[:],
                            relu_sb[:, kt, :, c0 : c0 + 128],
                            w2_sb[kt][:],
                            start=(kt == 0),
                            stop=(kt == 5),
                            perf_mode=DR,
                        )
                    ot = p_y.tile([128, E], BF16, tag="ot")
                    nc.vector.tensor_add(ot[:], ps[:], st["xb2"][it][tt][:])
                    nc.sync.dma_start(out_d[i, tt * 128 : (tt + 1) * 128, :], ot[:])

        n_pairs = IPC // 2
        sts = {}
        sts[0] = stage_a1(0)
        load_weights()
        load_biases()
        for k in range(1, min(3, n_pairs)):
            sts[k] = stage_a1(k)
        for k in range(min(2, n_pairs)):
            stage_a2(k, sts[k])
        stage_b1(0, sts[0])
        for pp in range(n_pairs):
            if pp + 3 < n_pairs:
                sts[pp + 3] = stage_a1(pp + 3)
            if pp + 2 < n_pairs:
                stage_a2(pp + 2, sts[pp + 2])
            if pp + 1 < n_pairs:
                stage_b1(pp + 1, sts[pp + 1])
            stage_b2a(pp, sts[pp])
            stage_b2b(pp, sts.pop(pp))

    nc.compile()
    return nc


def _prep_inputs(inputs):
    f = lambda v: np.asarray(v, dtype=np.float32)
    x = f(inputs["x"])
    Wq, Wk, Wv, Wo = f(inputs["Wq"]), f(inputs["Wk"]), f(inputs["Wv"]), f(inputs["Wo"])
    bo = f(inputs["bo"])
    W1, b1, W2, b2 = f(inputs["W1"]), f(inputs["b1"]), f(inputs["W2"]), f(inputs["b2"])
    g1, be1 = f(inputs["g1"]), f(inputs["be1"])
    g2, be2 = f(inputs["g2"]), f(inputs["be2"])

    scale = HS ** -0.5
    wq = (g1[:, None] * Wq) * scale
    bq = ((be1 @ Wq) * scale).astype(np.float32)
    wk = g1[:, None] * Wk
    bk = (be1 @ Wk).astype(np.float32)
    wv = g1[:, None] * Wv
    bv = be1 @ Wv
    # bv folded through Wo (softmax rows sum to 1): bo' = bv @ Wo + bo
    bo_p = (bv @ Wo + bo).astype(np.float32)
    w1 = g2[:, None] * W1
    b1_p = (be2 @ W1 + b1).astype(np.float32)

    def q8(a):
        return np.clip(a, -240.0, 240.0).astype(F8NP)

    def pack2(a):  # [256, M] -> [128, 2, M] DoubleRow planes
        return np.ascontiguousarray(a.reshape(2, 128, -1).transpose(1, 0, 2))

    col128 = lambda v: np.ascontiguousarray(v.reshape(-1, 128).T.astype(np.float32))
    row128 = lambda v: np.ascontiguousarray(
        np.broadcast_to(v.astype(np.float32), (128, v.shape[0]))
    )
    common = {
        # q/k/v prescaled x64 (fp8 normal range); undone by eviction scale
        "wq8": q8(pack2(wq[:256] * 64)), "wqb": (wq[256:] * 64).astype(BF),
        "wk8": q8(pack2(wk[:256] * 64)), "wkb": (wk[256:] * 64).astype(BF),
        "wv8": q8(pack2(wv[:256] * 64)), "wvb": (wv[256:] * 64).astype(BF),
        "wo": Wo.astype(BF),
        "w1f8": q8(pack2(w1[:256])), "w1b": w1[256:].astype(BF),
        "w2f8": q8(
            np.ascontiguousarray(W2.reshape(6, 2, 128, E).transpose(0, 2, 1, 3))
        ),
        "bq": col128(bq), "bk": col128(bk), "b1": col128(b1_p),
        "bo": row128(bo_p), "b2": row128(b2.astype(np.float32)),
        "maskt": np.triu(np.ones((128, 128), np.float32)).astype(BF),
        "ones64": np.ones((128, 64), np.float32).astype(BF),
    }
    in_maps = []
    for c in range(N_CORES):
        m = dict(common)
        m["x"] = np.ascontiguousarray(x[c * IPC : (c + 1) * IPC]).astype(BF)
        in_maps.append(m)
    return in_maps


def kernel(**inputs):
    if "nc" not in _CACHE:
        _CACHE["nc"] = _build()
    nc = _CACHE["nc"]
    in_maps = _prep_inputs(inputs)
    res = run_bass_kernel_spmd(nc, in_maps, core_ids=list(range(N_CORES)))
    _CACHE["last_result"] = res
    return np.concatenate([r["out"] for r in res.results], axis=0).astype(np.float32)



# revision 35
# speedup vs baseline: 1.2206x; 1.0495x over previous
"""Trainium2 Bass kernel for a dense transformer block.

Computes: ffwd(ln2(sa(ln1(x)) + x)) + x  (residual 2 connects to x)
with causal self-attention (6 heads, head_size 64), seq len 256, n_embed 384.

Sharding: data-parallel over batch (B=256) -> 32 items per NeuronCore,
weights replicated. All LN gains/biases and the softmax scale are folded
into the weight matrices on the host; matmul inputs are bf16 with fp32
PSUM accumulation; the LN/residual spine stays fp32.
"""

import sys
from contextlib import ExitStack

sys.path.insert(0, "/opt/trn_rl_repo")

import numpy as np
import ml_dtypes

import concourse.bass as bass
import concourse.tile as tile
from concourse import bacc, mybir
from concourse.bass_utils import run_bass_kernel_spmd

B, T, E, H, HS = 256, 256, 384, 6, 64
FF = 4 * E  # 1536
N_CORES = 8
IPC = B // N_CORES  # items per core
EPS = 1e-5

BF16 = mybir.dt.bfloat16
F32 = mybir.dt.float32
F8 = mybir.dt.float8e4
AF = mybir.ActivationFunctionType
OP = mybir.AluOpType
DR = mybir.MatmulPerfMode.DoubleRow
BF = ml_dtypes.bfloat16
F8NP = ml_dtypes.float8_e4m3

_CACHE = {}


def _setup_act_tables():
    """Force a single ACT table set (natural_log_exp_and_others) so walrus
    never thrashes between the exp and ln sets: we only use ln/exp/relu/
    copy/identity, which all live in that one set."""
    import os, json, tempfile

    if "BASS_ACT_ROOT_JSON_PATH" in os.environ:
        return
    from neuronxcc.driver.Job import Job
    from neuronxcc.driver.jobs.support.FindActInfo import findActInfoFile

    src = findActInfoFile(Job.getPackageDir(), "gen3")
    d = json.load(open(src))
    keep = [s for s in d["act_func_sets"] if s["name"] == "natural_log_exp_and_others"]
    assert keep, "natural_log_exp_and_others set not found"
    d["act_func_sets"] = keep
    dst_dir = tempfile.mkdtemp(prefix="act_custom_")
    srcdir = os.path.dirname(src)
    for key in d["pwp_file_keys"]:
        fn = keep[0][key]
        os.symlink(os.path.join(srcdir, fn), os.path.join(dst_dir, fn))
    dst = os.path.join(dst_dir, "act_info.json")
    with open(dst, "w") as f:
        json.dump(d, f)
    os.environ["BASS_ACT_ROOT_JSON_PATH"] = dst

    # Bacc's insert_act_table_loads must agree with walrus on set ids:
    # filter its table view to the same single set (id 0).
    import concourse.hw_specs as hw_specs
    import concourse.bacc as bacc_mod

    orig = hw_specs.get_activation_tables

    def filtered(arch):
        t = orig(arch)
        return {"natural_log_exp_and_others": t["natural_log_exp_and_others"]}

    hw_specs.get_activation_tables = filtered
    bacc_mod.get_activation_tables = filtered


def _build():
    _setup_act_tables()
    nc = bacc.Bacc("TRN2", target_bir_lowering=False, debug=False)

    x_d = nc.dram_tensor("x", [IPC, T, E], BF16, kind="ExternalInput").ap()
    # q/k/v/ffn1 weights: features 0-255 as fp8 DoubleRow planes, 256-383 bf16
    wq8_d = nc.dram_tensor("wq8", [128, 2, E], F8, kind="ExternalInput").ap()
    wk8_d = nc.dram_tensor("wk8", [128, 2, E], F8, kind="ExternalInput").ap()
    wv8_d = nc.dram_tensor("wv8", [128, 2, E], F8, kind="ExternalInput").ap()
    wqb_d = nc.dram_tensor("wqb", [128, E], BF16, kind="ExternalInput").ap()
    wkb_d = nc.dram_tensor("wkb", [128, E], BF16, kind="ExternalInput").ap()
    wvb_d = nc.dram_tensor("wvb", [128, E], BF16, kind="ExternalInput").ap()
    wo_d = nc.dram_tensor("wo", [E, E], BF16, kind="ExternalInput").ap()
    w1f8_d = nc.dram_tensor("w1f8", [128, 2, FF], F8, kind="ExternalInput").ap()
    w1b_d = nc.dram_tensor("w1b", [128, FF], BF16, kind="ExternalInput").ap()
    w2f8_d = nc.dram_tensor("w2f8", [6, 128, 2, E], F8, kind="ExternalInput").ap()
    # biases pre-laid-out on the host: [128, n] column tiles / [128, E] rows
    bq_d = nc.dram_tensor("bq", [128, 3], F32, kind="ExternalInput").ap()
    bk_d = nc.dram_tensor("bk", [128, 3], F32, kind="ExternalInput").ap()
    b1_d = nc.dram_tensor("b1", [128, 12], F32, kind="ExternalInput").ap()
    bo_d = nc.dram_tensor("bo", [128, E], F32, kind="ExternalInput").ap()
    b2_d = nc.dram_tensor("b2", [128, E], F32, kind="ExternalInput").ap()
    mask_d = nc.dram_tensor("maskt", [128, 128], BF16, kind="ExternalInput").ap()
    ones_d = nc.dram_tensor("ones64", [128, 64], BF16, kind="ExternalInput").ap()
    out_d = nc.dram_tensor("out", [IPC, T, E], BF16, kind="ExternalOutput").ap()

    with tile.TileContext(nc) as tc, ExitStack() as ctx:
        singles = ctx.enter_context(tc.tile_pool(name="singles", bufs=1))
        p_x = ctx.enter_context(tc.tile_pool(name="p_x", bufs=16))
        p_h = ctx.enter_context(tc.tile_pool(name="p_h", bufs=8))
        p_T = ctx.enter_context(tc.tile_pool(name="p_T", bufs=2))
        p_h1T = ctx.enter_context(tc.tile_pool(name="p_h1T", bufs=4))
        p_relu = ctx.enter_context(tc.tile_pool(name="p_relu", bufs=2))
        p_xb = ctx.enter_context(tc.tile_pool(name="p_xb", bufs=8))
        p_rec = ctx.enter_context(tc.tile_pool(name="p_rec", bufs=4))
        p_qk = ctx.enter_context(tc.tile_pool(name="p_qk", bufs=3))
        p_e = ctx.enter_context(tc.tile_pool(name="p_e", bufs=2))
        p_sm = ctx.enter_context(tc.tile_pool(name="p_sm", bufs=16))
        p_y = ctx.enter_context(tc.tile_pool(name="p_y", bufs=6))

        pp_big = ctx.enter_context(tc.tile_pool(name="pp_big", bufs=2, space="PSUM"))
        pp_v = ctx.enter_context(tc.tile_pool(name="pp_v", bufs=2, space="PSUM"))
        pp_s = ctx.enter_context(tc.tile_pool(name="pp_s", bufs=2, space="PSUM"))
        pp_do = ctx.enter_context(tc.tile_pool(name="pp_do", bufs=2, space="PSUM"))

        # ---- constants / weights (emission deferred via load_weights so the
        # first pair's x DMAs + LN1 go out ahead of the bulk weight traffic) ----
        w8_sb = {}
        wb_sb = {}
        wo_sb = []
        w2_sb = []

        def load_weights():
            # qkv first (stage_a2(0) needs them soonest), then wo/w1/w2
            for nm, src8, srcb in (
                ("q", wq8_d, wqb_d), ("k", wk8_d, wkb_d), ("v", wv8_d, wvb_d)
            ):
                t8 = singles.tile([128, 2, E], F8, tag=f"w8_{nm}")
                nc.scalar.dma_start(t8[:], src8[:])
                w8_sb[nm] = t8
                tb = singles.tile([128, E], BF16, tag=f"wb_{nm}")
                nc.scalar.dma_start(tb[:], srcb[:])
                wb_sb[nm] = tb
            for kt in range(3):
                t = singles.tile([128, E], BF16, tag=f"wo_{kt}")
                nc.scalar.dma_start(t[:], wo_d[kt * 128 : (kt + 1) * 128, :])
                wo_sb.append(t)
            t8 = singles.tile([128, 2, FF], F8, tag="w8_1")
            nc.scalar.dma_start(t8[:], w1f8_d[:])
            w8_sb["1"] = t8
            tb = singles.tile([128, FF], BF16, tag="wb_1")
            nc.scalar.dma_start(tb[:], w1b_d[:])
            wb_sb["1"] = tb
            for kt in range(6):
                t = singles.tile([128, 2, E], F8, tag=f"w2_{kt}")
                nc.scalar.dma_start(t[:], w2f8_d[kt])
                w2_sb.append(t)

        bq_sb = singles.tile([128, 3], F32, tag="bq")
        bk_sb = singles.tile([128, 3], F32, tag="bk")
        b1_sb = singles.tile([128, 12], F32, tag="b1")
        bo_bc = singles.tile([128, E], F32, tag="bo_bc")
        b2_bc = singles.tile([128, E], F32, tag="b2_bc")
        maskt = singles.tile([128, 128], BF16, tag="maskt")
        ones64 = singles.tile([128, 64], BF16, tag="ones64")

        def load_biases():
            for t, srcd in ((bq_sb, bq_d), (bk_sb, bk_d), (b1_sb, b1_d),
                            (bo_bc, bo_d), (b2_bc, b2_d), (maskt, mask_d),
                            (ones64, ones_d)):
                nc.scalar.dma_start(t[:], srcd[:])

        eps_t = singles.tile([128, 1], F32, tag="eps")
        nc.vector.memset(eps_t[:], EPS)
        warm_act = singles.tile([128, 1], F32, tag="warm_act")
        nc.scalar.activation(warm_act[:], eps_t[:], AF.Exp)

        def layernorm_quad(pairs):
            """For each (src, dst) in pairs (up to 4): dst (bf16) =
            (src - mean) * rsqrt(var + eps) row-wise over 384.
            The rsqrt is exp(-0.5*ln(var+eps)) batched over all tiles so the
            per-op ACT overhead is paid once, and only the exp/ln table set
            is ever touched."""
            n = len(pairs)
            mv_all = p_sm.tile([128, n, 2], F32, tag="bnmv")
            for j, (src, _) in enumerate(pairs):
                st = p_sm.tile([128, 6], F32, tag="bnst")
                nc.vector.bn_stats(st[:], src[:])
                nc.vector.bn_aggr(mv_all[:, j, :], st[:])
            lnv = p_sm.tile([128, n], F32, tag="lnv")
            nc.scalar.activation(lnv[:], mv_all[:, :, 1], AF.Ln, bias=eps_t[:])
            rstd = p_sm.tile([128, n], F32, tag="rstd")
            nc.scalar.activation(rstd[:], lnv[:], AF.Exp, scale=-0.5)
            for j, (src, dst) in enumerate(pairs):
                nc.vector.tensor_scalar(
                    dst[:], src[:], mv_all[:, j, 0:1], rstd[:, j : j + 1],
                    op0=OP.subtract, op1=OP.mult,
                )

        # ---- main loop: 16 pairs of batch items, software-pipelined ----
        # Stage A (load, LN1, transpose, QKV) of pair pp+1 is emitted before
        # stage B (attention, sa, LN2, FFN, out) of pair pp so the PE always
        # has independent matmul work during B's LN2 serial chain (keeps the
        # HAM clock gate warm).

        def stage_a1(pp):
            x_sb = [[None, None], [None, None]]
            h1_sb = [[None, None], [None, None]]
            ln_pairs = []
            for it in range(2):
                i = 2 * pp + it
                for tt in range(2):
                    xt = p_x.tile([128, E], BF16, tag="x")
                    nc.sync.dma_start(
                        xt[:], x_d[i, tt * 128 : (tt + 1) * 128, :]
                    )
                    x_sb[it][tt] = xt
                    h1 = p_h.tile([128, E], BF16, tag="h1")
                    ln_pairs.append((xt, h1))
                    h1_sb[it][tt] = h1
            layernorm_quad(ln_pairs)

            # transpose h1 -> h1T [E, 2*T] (feature-major) via DMA xbar
            h1T = p_h1T.tile([128, 3, 512], BF16, tag="h1T")
            for it in range(2):
                for tt in range(2):
                    c0 = it * 256 + tt * 128
                    nc.sync.dma_start_transpose(
                        out=h1T[:, :, c0 : c0 + 128], in_=h1_sb[it][tt][:]
                    )
            # fp8 copy of feature planes 0,1 for the DoubleRow matmuls
            h1T8 = p_h1T.tile([128, 2, 512], F8, tag="h1T8")
            nc.vector.tensor_copy(h1T8[:], h1T[:, 0:2, :])
            return dict(x_sb=x_sb, h1T=h1T, h1T8=h1T8)

        def stage_a2(pp, st):
            h1T, h1T8 = st["h1T"], st["h1T8"]
            # qT, kT projections (feature-major): qT[f, t] over both items.
            # Weights are prescaled x64 on the host (fp8 normal range); the
            # eviction's free activation scale undoes it exactly.
            qT = p_qk.tile([128, 3, 512], BF16, tag="qT")
            kT = p_qk.tile([128, 3, 512], BF16, tag="kT")
            for dst, w8, wb, b_sb in (
                (qT, w8_sb["q"], wb_sb["q"], bq_sb),
                (kT, w8_sb["k"], wb_sb["k"], bk_sb),
            ):
                for ft in range(3):
                    ps = pp_big.tile([128, 512], F32, tag="ps_big")
                    nc.tensor.matmul(
                        ps[:], wb[:, ft * 128 : (ft + 1) * 128], h1T[:, 2, :],
                        start=True, stop=False,
                    )
                    nc.tensor.matmul(
                        ps[:], w8[:, :, ft * 128 : (ft + 1) * 128], h1T8[:],
                        start=False, stop=True, perf_mode=DR,
                    )
                    nc.scalar.activation(
                        dst[:, ft, :], ps[:], AF.Identity,
                        bias=b_sb[:, ft : ft + 1], scale=1.0 / 64,
                    )

            # v (token-major): v[t, f] per item
            v_sb = [None, None]
            for it in range(2):
                vt = p_h.tile([128, 2, E], BF16, tag="v")
                for tt in range(2):
                    c0 = it * 256 + tt * 128
                    ps = pp_v.tile([128, E], F32, tag="ps_v")
                    nc.tensor.matmul(
                        ps[:], h1T[:, 2, c0 : c0 + 128], wb_sb["v"][:],
                        start=True, stop=False,
                    )
                    nc.tensor.matmul(
                        ps[:], h1T8[:, :, c0 : c0 + 128], w8_sb["v"][:],
                        start=False, stop=True, perf_mode=DR,
                    )
                    nc.scalar.activation(vt[:, tt, :], ps[:], AF.Identity, scale=1.0 / 64)
                v_sb[it] = vt
            st["qT"], st["kT"], st["v_sb"] = qT, kT, v_sb

        def stage_b1(pp, st):
            qT, kT, v_sb = st["qT"], st["kT"], st["v_sb"]
            # residual bases with biases folded in (on the idle GPSIMD):
            # xbo = x + bo' (feeds y), xb2 = x + b2 (feeds final out)
            xbo = [[None, None], [None, None]]
            xb2 = [[None, None], [None, None]]
            for it in range(2):
                for tt in range(2):
                    a = p_xb.tile([128, E], F32, tag="xbo")
                    nc.gpsimd.tensor_add(a[:], st["x_sb"][it][tt][:], bo_bc[:])
                    xbo[it][tt] = a
                    b = p_xb.tile([128, E], F32, tag="xb2")
                    nc.gpsimd.tensor_add(b[:], st["x_sb"][it][tt][:], b2_bc[:])
                    xb2[it][tt] = b
            st["xbo"], st["xb2"] = xbo, xb2
            # attention, transposed layout: scoresT[k, q] per (item, head)
            attnT = p_T.tile([128, 3, 512], BF16, tag="attnT")
            for it in range(2):
                c0 = it * 256
                expA = p_e.tile([128, 6, 384], BF16, tag="expA")
                for h in range(H):
                    p3 = h // 2
                    r0 = 64 * (h % 2)
                    qh = qT[r0 : r0 + 64, p3, c0 : c0 + 256]
                    ps_s = pp_s.tile([128, 384], F32, tag="ps_s")
                    # k-tile 0: all q columns
                    nc.tensor.matmul(
                        ps_s[:, 0:256],
                        kT[r0 : r0 + 64, p3, c0 : c0 + 128],
                        qh,
                        start=True,
                        stop=True,
                        skip_group_check=True,
                    )
                    # k-tile 1: only q columns 128:256 (rest fully masked)
                    nc.tensor.matmul(
                        ps_s[:, 256:384],
                        kT[r0 : r0 + 64, p3, c0 + 128 : c0 + 256],
                        qT[r0 : r0 + 64, p3, c0 + 128 : c0 + 256],
                        start=True,
                        stop=True,
                        skip_group_check=True,
                    )
                    # one fused exp over both score blocks
                    nc.scalar.activation(expA[:, h, :], ps_s[:], AF.Exp)
                    if h % 2 == 1:
                        # causal masks for this head pair: diag block of
                        # k-tile 0 (q 0:128) and the k-tile 1 block
                        # (on GpSimd: frees DVE for the LN2/eviction chains)
                        hp = h // 2
                        nc.vector.tensor_tensor(
                            expA[:, 2 * hp : 2 * hp + 2, 0:128],
                            expA[:, 2 * hp : 2 * hp + 2, 0:128],
                            maskt[:, None, :].to_broadcast((128, 2, 128)),
                            op=OP.mult,
                        )
                        nc.vector.tensor_tensor(
                            expA[:, 2 * hp : 2 * hp + 2, 256:384],
                            expA[:, 2 * hp : 2 * hp + 2, 256:384],
                            maskt[:, None, :].to_broadcast((128, 2, 128)),
                            op=OP.mult,
                        )

                # head pairs (2h, 2h+1) share a partition tile: pack their
                # denominator and output matmuls into one [128, 256] psum via
                # col tile_position, halving the DVE normalize work.
                for hp in range(3):
                    ps_d = pp_do.tile([128, 256], F32, tag="ps_do")
                    ps_o = pp_do.tile([128, 256], F32, tag="ps_do")
                    for sub in range(2):
                        h = 2 * hp + sub
                        r0 = 64 * sub
                        tp = (0, r0)
                        e0 = expA[:, h, 0:256]
                        e1 = expA[:, h, 256:384]
                        nc.tensor.matmul(
                            ps_d[r0 : r0 + 64, :], ones64[:], e0,
                            start=True, stop=False,
                            skip_group_check=True, tile_position=tp,
                        )
                        nc.tensor.matmul(
                            ps_d[r0 : r0 + 64, 128:256], ones64[:], e1,
                            start=False, stop=True,
                            skip_group_check=True, tile_position=tp,
                        )
                        nc.tensor.matmul(
                            ps_o[r0 : r0 + 64, :],
                            v_sb[it][:, 0, 64 * h : 64 * h + 64], e0,
                            start=True, stop=False,
                            skip_group_check=True, tile_position=tp,
                        )
                        nc.tensor.matmul(
                            ps_o[r0 : r0 + 64, 128:256],
                            v_sb[it][:, 1, 64 * h : 64 * h + 64], e1,
                            start=False, stop=True,
                            skip_group_check=True, tile_position=tp,
                        )
                    rec = p_rec.tile([128, 256], F32, tag="rec")
                    nc.vector.reciprocal_approx_fast(rec[:], ps_d[:])
                    nc.vector.tensor_tensor(
                        attnT[:, hp, c0 : c0 + 256], ps_o[:], rec[:], op=OP.mult,
                    )

            st["attnT"] = attnT

        def warm(n=1):
            for _ in range(n):
                pw = pp_v.tile([64, 384], F32, tag="ps_v")
                nc.tensor.matmul(
                    pw[:], ones64[:], wo_sb[0][:], start=True, stop=True,
                    skip_group_check=True,
                )

        def stage_b2a(pp, st):
            attnT = st["attnT"]
            # sa (token-major) + residual + bo, then LN2
            h2_sb = [[None, None], [None, None]]
            ln2_pairs = []
            for it in range(2):
                for tt in range(2):
                    ps = pp_v.tile([128, E], F32, tag="ps_v")
                    for kt in range(3):
                        nc.tensor.matmul(
                            ps[:],
                            attnT[:, kt, it * 256 + tt * 128 : it * 256 + tt * 128 + 128],
                            wo_sb[kt][:],
                            start=(kt == 0),
                            stop=(kt == 2),
                        )
                    y = p_y.tile([128, E], F32, tag="y")
                    nc.vector.tensor_add(y[:], ps[:], st["xbo"][it][tt][:])
                    h2 = p_h.tile([128, E], BF16, tag="h2")
                    ln2_pairs.append((y, h2))
                    h2_sb[it][tt] = h2

            layernorm_quad(ln2_pairs)
            warm(3)

            # transpose h2 -> h2T via DMA xbar (PE transpose-mode does not
            # count as HAM activity and was re-throttling the PE clock)
            h2T = p_T.tile([128, 3, 512], BF16, tag="h2T")
            h2T8 = p_T.tile([128, 2, 512], F8, tag="h2T8")
            # transposes split across both HWDGE queues (they serialize
            # ~1.25us each per queue); fp8 quantize per item half so FFN1's
            # item-0 matmuls start a transpose-pair earlier
            for it in range(2):
                c0 = it * 256
                nc.sync.dma_start_transpose(
                    out=h2T[:, :, c0 : c0 + 128], in_=h2_sb[it][0][:]
                )
                nc.scalar.dma_start_transpose(
                    out=h2T[:, :, c0 + 128 : c0 + 256], in_=h2_sb[it][1][:]
                )
                nc.vector.tensor_copy(
                    h2T8[:, :, c0 : c0 + 256], h2T[:, 0:2, c0 : c0 + 256]
                )
            st["h2T"], st["h2T8"] = h2T, h2T8

        def stage_b2b(pp, st):
            h2T, h2T8 = st["h2T"], st["h2T8"]
            # FFN: reluT[c, t] = relu(W1'^T @ h2T + b1'), feature-major.
            # fp8 DoubleRow over features 0-255 + bf16 tail; w1/w2 quantized
            # at natural scale so both eviction engines need no rescale.
            relu_sb = p_relu.tile([128, 6, 2, 512], F8, tag="relu")
            for mt in range(12):
                ps = pp_big.tile([128, 512], F32, tag="ps_big")
                for it in range(2):
                    c0 = it * 256
                    nc.tensor.matmul(
                        ps[:, c0 : c0 + 256],
                        wb_sb["1"][:, mt * 128 : (mt + 1) * 128],
                        h2T[:, 2, c0 : c0 + 256],
                        start=True, stop=False,
                        skip_group_check=True,
                    )
                    nc.tensor.matmul(
                        ps[:, c0 : c0 + 256],
                        w8_sb["1"][:, :, mt * 128 : (mt + 1) * 128],
                        h2T8[:, :, c0 : c0 + 256],
                        start=False, stop=True, perf_mode=DR,
                        skip_group_check=True,
                    )
                dst = relu_sb[:, mt // 2, mt % 2, :]
                if mt % 2 == 0:
                    nc.scalar.activation(
                        dst, ps[:], AF.Relu, bias=b1_sb[:, mt : mt + 1]
                    )
                else:
                    # split evictions across ScalarE/DVE: the serial relu
                    # chain paces FFN2 otherwise
                    nc.vector.tensor_scalar(
                        dst, ps[:], b1_sb[:, mt : mt + 1], 0.0,
                        op0=OP.add, op1=OP.max,
                    )

            # ff (token-major) + residual to x + b2, DMA out
            for it in range(2):
                i = 2 * pp + it
                for tt in range(2):
                    c0 = it * 256 + tt * 128
                    ps = pp_v.tile([128, E], F32, tag="ps_v")
                    for kt in range(6):
                        nc.tensor.matmul(
                            ps[:],
                            relu_sb[:, kt, :, c0 : c0 + 128],
                            w2_sb[kt][:],
                            start=(kt == 0),
                            stop=(kt == 5),
                            perf_mode=DR,
                        )
                    ot = p_y.tile([128, E], BF16, tag="ot")
                    nc.vector.tensor_add(ot[:], ps[:], st["xb2"][it][tt][:])
                    nc.sync.dma_start(out_d[i, tt * 128 : (tt + 1) * 128, :], ot[:])

        n_pairs = IPC // 2
        sts = {}
        sts[0] = stage_a1(0)
        load_weights()
        load_biases()
        for k in range(1, min(3, n_pairs)):
            sts[k] = stage_a1(k)
        for k in range(min(2, n_pairs)):
            stage_a2(k, sts[k])
        stage_b1(0, sts[0])
        for pp in range(n_pairs):
            if pp + 3 < n_pairs:
                sts[pp + 3] = stage_a1(pp + 3)
            if pp + 2 < n_pairs:
                stage_a2(pp + 2, sts[pp + 2])
            if pp + 1 < n_pairs:
                stage_b1(pp + 1, sts[pp + 1])
            stage_b2a(pp, sts[pp])
            stage_b2b(pp, sts.pop(pp))

    nc.compile()
    return nc


def _prep_inputs(inputs):
    f = lambda v: np.asarray(v, dtype=np.float32)
    x = f(inputs["x"])
    Wq, Wk, Wv, Wo = f(inputs["Wq"]), f(inputs["Wk"]), f(inputs["Wv"]), f(inputs["Wo"])
    bo = f(inputs["bo"])
    W1, b1, W2, b2 = f(inputs["W1"]), f(inputs["b1"]), f(inputs["W2"]), f(inputs["b2"])
    g1, be1 = f(inputs["g1"]), f(inputs["be1"])
    g2, be2 = f(inputs["g2"]), f(inputs["be2"])

    scale = HS ** -0.5
    wq = (g1[:, None] * Wq) * scale
    bq = ((be1 @ Wq) * scale).astype(np.float32)
    wk = g1[:, None] * Wk
    bk = (be1 @ Wk).astype(np.float32)
    wv = g1[:, None] * Wv
    bv = be1 @ Wv
    # bv folded through Wo (softmax rows sum to 1): bo' = bv @ Wo + bo
    bo_p = (bv @ Wo + bo).astype(np.float32)
    w1 = g2[:, None] * W1
    b1_p = (be2 @ W1 + b1).astype(np.float32)

    def q8(a):
        return np.clip(a, -240.0, 240.0).astype(F8NP)

    def pack2(a):  # [256, M] -> [128, 2, M] DoubleRow planes
        return np.ascontiguousarray(a.reshape(2, 128, -1).transpose(1, 0, 2))

    col128 = lambda v: np.ascontiguousarray(v.reshape(-1, 128).T.astype(np.float32))
    row128 = lambda v: np.ascontiguousarray(
        np.broadcast_to(v.astype(np.float32), (128, v.shape[0]))
    )
    common = {
        # q/k/v prescaled x64 (fp8 normal range); undone by eviction scale
        "wq8": q8(pack2(wq[:256] * 64)), "wqb": (wq[256:] * 64).astype(BF),
        "wk8": q8(pack2(wk[:256] * 64)), "wkb": (wk[256:] * 64).astype(BF),
        "wv8": q8(pack2(wv[:256] * 64)), "wvb": (wv[256:] * 64).astype(BF),
        "wo": Wo.astype(BF),
        "w1f8": q8(pack2(w1[:256])), "w1b": w1[256:].astype(BF),
        "w2f8": q8(
            np.ascontiguousarray(W2.reshape(6, 2, 128, E).transpose(0, 2, 1, 3))
        ),
        "bq": col128(bq), "bk": col128(bk), "b1": col128(b1_p),
        "bo": row128(bo_p), "b2": row128(b2.astype(np.float32)),
        "maskt": np.triu(np.ones((128, 128), np.float32)).astype(BF),
        "ones64": np.ones((128, 64), np.float32).astype(BF),
    }
    in_maps = []
    for c in range(N_CORES):
        m = dict(common)
        m["x"] = np.ascontiguousarray(x[c * IPC : (c + 1) * IPC]).astype(BF)
        in_maps.append(m)
    return in_maps


def kernel(**inputs):
    if "nc" not in _CACHE:
        _CACHE["nc"] = _build()
    nc = _CACHE["nc"]
    in_maps = _prep_inputs(inputs)
    res = run_bass_kernel_spmd(nc, in_maps, core_ids=list(range(N_CORES)))
    _CACHE["last_result"] = res
    return np.concatenate([r["out"] for r in res.results], axis=0).astype(np.float32)

